# revision 31
# baseline (speedup 1.0000x reference)
"""Trainium2 Bass kernel for a dense transformer block (LN->attn->LN->MLP).

Sharding: 8 cores = (batch b in 0..3, parity h in 0..1). Core (b,h) owns the
interleaved 128-row q-blocks {h, h+2, ...} of batch b.  Host permutes the
batch's rows so the core's own blocks come first; causal structure is then
identical on every core (uniform SPMD program): own q-block i attends to
permuted kv-blocks [0..i] (own parity, triangular mask on block i) and
[NOB..NOB+i] (other parity; parity-dependent masking supplied as per-core
mask data).

Attention (v3): q-blocks processed in groups of 256 columns; S^T[kv,q] tiles
for two kv-blocks share one PSUM bank and one ACT exp; AV computes
av'^T = V'^T @ P^T (lhsT = V' with appended ones column -> softmax sums in
row 64), then a short transpose chain normalizes and re-transposes per
128-q-half.  All matmuls are bf16 (fp32 matmul runs 2-pass LOW_HIGH);
accumulation stays fp32 in PSUM, LN/softmax/residual arithmetic fp32.
"""

import math
import threading
from contextlib import ExitStack

import numpy as np

import concourse.bass as bass
import concourse.mybir as mybir
import concourse.tile as tile
from concourse import bacc, bass_utils
from concourse.masks import (make_identity, make_lower_triangular,
                             make_upper_triangular)

AF = mybir.ActivationFunctionType
OP = mybir.AluOpType
DT = mybir.dt.float32
BF = mybir.dt.bfloat16
F8 = mybir.dt.float8e4
PM = mybir.MatmulPerfMode
W8_SCALE = 64.0

LN_EPS = 1e-5
MASK_VAL = -30000.0



def build_block_program(T=2048, C=1024, H=16, gelu_mode="hw",
                        mm_dtype="bf16", skip=()):
    """Build the per-core SPMD Bass program. Returns compiled Bacc.

    skip: subset of {"qkv_bias","o_bias","fc_bias","proj_bias","ln1_gb",
    "ln2_gb"} -- ops elided because the host verified the params are
    identity (zero bias / unit gain).
    """
    D = 64
    GELU_C = math.sqrt(2.0 / math.pi)
    MT = BF if mm_dtype == "bf16" else DT
    NB = T // 128            # kv blocks (permuted)
    NOB = NB // 2            # own q-blocks
    NOG = NOB // 2           # own q-groups (256 rows)
    OWN = NOB * 128          # own rows
    NCCH = C // 128          # feature chunks
    F = 4 * C
    NF = F // 128
    HP = H // 2              # head pairs
    HPG = HP                 # single pass: all head pairs resident
    W1 = HPG * 128           # qkv weight tile width
    BN_W = min(C, 512)       # bn_stats subgroup width
    NST = C // BN_W

    GSZ = min(512, OWN)
    kv_groups = [(g, min(512, T - g)) for g in range(0, T, 512)]
    own_groups = [(g, min(GSZ, OWN - g)) for g in range(0, OWN, GSZ)]

    nc = bacc.Bacc("TRN2", target_bir_lowering=False, debug=False)

    xk = nc.dram_tensor("xk", [T, C], DT, kind="ExternalInput")
    maskq = nc.dram_tensor("maskq", [128, 2, 256], MT, kind="ExternalInput")
    Wq = nc.dram_tensor("Wq", [C, C], MT, kind="ExternalInput")
    Wk = nc.dram_tensor("Wk", [C, C], MT, kind="ExternalInput")
    Wv = nc.dram_tensor("Wv", [C, C], MT, kind="ExternalInput")
    Wo = nc.dram_tensor("Wo", [C, C], MT, kind="ExternalInput")
    bq = nc.dram_tensor("bq", [C], DT, kind="ExternalInput")
    bk = nc.dram_tensor("bk", [C], DT, kind="ExternalInput")
    bv = nc.dram_tensor("bv", [C], DT, kind="ExternalInput")
    bo = nc.dram_tensor("bo", [C], DT, kind="ExternalInput")
    ln1_g = nc.dram_tensor("ln1_g", [C], DT, kind="ExternalInput")
    ln1_b = nc.dram_tensor("ln1_b", [C], DT, kind="ExternalInput")
    ln2_g = nc.dram_tensor("ln2_g", [C], DT, kind="ExternalInput")
    ln2_b = nc.dram_tensor("ln2_b", [C], DT, kind="ExternalInput")
    W_fc = nc.dram_tensor("W_fc", [C, F], MT, kind="ExternalInput")
    b_fc = nc.dram_tensor("b_fc", [F], DT, kind="ExternalInput")
    W_proj = nc.dram_tensor("W_proj", [F, C], MT, kind="ExternalInput")
    b_proj = nc.dram_tensor("b_proj", [C], DT, kind="ExternalInput")
    out = nc.dram_tensor("out", [OWN, C], DT, kind="ExternalOutput")

    with tile.TileContext(nc) as tc:
        with ExitStack() as es0:
            consts = es0.enter_context(tc.tile_pool(name="consts", bufs=1))
            persist = es0.enter_context(tc.tile_pool(name="persist", bufs=1))
            dram = es0.enter_context(
                tc.tile_pool(name="dram", bufs=1, space="DRAM"))
            identity_m = consts.tile([128, 128], MT)
            make_identity(nc, identity_m)
            identity = consts.tile([128, 128], DT)
            make_identity(nc, identity)
            # multiplicative diag mask for own-parity block pair (2g, 2g+1):
            # [triu1 | ones | zeros | triu1] over S^T tiles [kv, q]
            mD = consts.tile([128, 512], MT)
            make_upper_triangular(nc, mD[:, 0:128], val=1.0, diag=True)
            nc.gpsimd.memset(mD[:, 128:256], 1.0)
            nc.gpsimd.memset(mD[:, 256:384], 0.0)
            make_upper_triangular(nc, mD[:, 384:512], val=1.0, diag=True)
            # per-core multiplicative parity masks (0/1), [slot0|slot1]
            mq = consts.tile([128, 512], MT)
            nc.gpsimd.dma_start(out=mq, in_=maskq[:, :, :])
            eps_t = consts.tile([128, 1], DT)
            nc.vector.memset(eps_t, LN_EPS)
            # bf16 ones row (softmax-denominator outer-product broadcast)
            onesb = consts.tile([1, 65], MT)
            nc.vector.memset(onesb, 1.0)
            # PE warmup: keep the systolic array busy through the initial
            # DMA window so the HAM clock gate opens before real matmuls
            wz = consts.tile([128, 512], MT)
            nc.gpsimd.memset(wz, 0.0)
            with ExitStack() as eswu:
                wup = eswu.enter_context(
                    tc.tile_pool(name="wup", bufs=1, space="PSUM"))
                for _ in range(40):
                    wps = wup.tile([128, 512], DT, tag="wps", name="wps")
                    nc.tensor.matmul(wps, identity_m, wz, start=True,
                                     stop=True)

            def bcast_tile(vec):
                t = consts.tile([128, C], DT, tag=f"bc_{vec.name}",
                                name=f"bc_{vec.name}")
                src = bass.AP(tensor=vec, offset=0, ap=[[0, 128], [1, C]])
                nc.gpsimd.dma_start(out=t, in_=src)
                return t

            ln1g_t = bcast_tile(ln1_g) if "ln1_gb" not in skip else None
            ln1b_t = bcast_tile(ln1_b) if "ln1_gb" not in skip else None
            ln2g_t = bcast_tile(ln2_g) if "ln2_gb" not in skip else None
            ln2b_t = bcast_tile(ln2_b) if "ln2_gb" not in skip else None

            def chunk_tile(vec, n):
                t = consts.tile([128, n], DT, tag=f"ck_{vec.name}",
                                name=f"ck_{vec.name}")
                nc.gpsimd.dma_start(
                    out=t, in_=vec.ap().rearrange("(a p) -> p a", p=128))
                return t

            qkv_bias = "qkv_bias" not in skip
            bq_t = chunk_tile(bq, NCCH) if qkv_bias else None
            bvb_t = bcast_tile(bv) if qkv_bias else None
            bk_t = chunk_tile(bk, NCCH) if qkv_bias else None
            bv_t = chunk_tile(bv, NCCH) if qkv_bias else None
            bo_t = chunk_tile(bo, NCCH) if "o_bias" not in skip else None
            bfc_t = chunk_tile(b_fc, NF) if "fc_bias" not in skip else None
            bpr_t = chunk_tile(b_proj, NCCH) \
                if "proj_bias" not in skip else None

            qT_d = dram.tile([HP, 128, OWN], MT, tag="qT", name="qT_d")
            avT_d = dram.tile([HP, 128, OWN], MT, tag="avT", name="avT_d")
            x2_d = dram.tile([OWN, C], DT, tag="x2", name="x2_d")

            def layernorm(pool, spool, xt, g_t, b_t, skip_gb):
                """LN of xt [128,C] f32 -> new MT tile."""
                stats = spool.tile([128, NST, 6], DT, tag="stats",
                                   name="stats")
                mv = spool.tile([128, 2], DT, tag="mv", name="mv")
                for s in range(NST):
                    nc.vector.bn_stats(out=stats[:, s, :],
                                       in_=xt[:, s * BN_W:(s + 1) * BN_W])
                nc.vector.bn_aggr(out=mv, in_=stats)
                rstd = spool.tile([128, 1], DT, tag="rstd", name="rstd")
                nc.scalar.activation(out=rstd, in_=mv[:, 1:2],
                                     func=AF.Sqrt, bias=eps_t[:, :])
                nc.vector.reciprocal(out=rstd, in_=rstd)
                ln_m = pool.tile([128, C], MT, tag="ln_m", name="ln_m")
                if skip_gb:
                    nc.vector.tensor_scalar(
                        out=ln_m, in0=xt, scalar1=mv[:, 0:1], scalar2=rstd,
                        op0=OP.subtract, op1=OP.mult)
                else:
                    ln = pool.tile([128, C], DT, tag="ln", name="ln")
                    nc.vector.tensor_scalar(
                        out=ln, in0=xt, scalar1=mv[:, 0:1], scalar2=rstd,
                        op0=OP.subtract, op1=OP.mult)
                    nc.vector.tensor_tensor(out=ln, in0=ln, in1=g_t,
                                            op=OP.mult)
                    nc.vector.tensor_tensor(out=ln_m, in0=ln, in1=b_t,
                                            op=OP.add)
                return ln_m

            # ===== Phases 1+2 per head-group =====
            for hg in range(HP // HPG):
                pairs = list(range(hg * HPG, (hg + 1) * HPG))
                hgs = slice(hg * W1, (hg + 1) * W1)
                with ExitStack() as es1:
                    kvp = es1.enter_context(tc.tile_pool(name="kvp", bufs=1))
                    KT = {p: kvp.tile([128, T], MT, tag=f"kt{p}",
                                      name=f"kt{p}") for p in pairs}
                    # V in [kv, head, dim|ones] layout, built directly by
                    # kv-block-major matmuls (lnT stationary, Wv moving) --
                    # no per-head transposes
                    VPA = kvp.tile([128, NB, H, 65], MT, tag="vpa",
                                   name="vpa")
                    nc.gpsimd.memset(VPA[:, :, :, 64:65], 1.0)
                    es1b = es1.enter_context(ExitStack())
                    p1sb = es1b.enter_context(
                        tc.tile_pool(name="p1sb", bufs=2))
                    p1st = es1b.enter_context(
                        tc.tile_pool(name="p1st", bufs=8))
                    p1lt = es1b.enter_context(
                        tc.tile_pool(name="p1lt", bufs=2))
                    p1w = es1b.enter_context(
                        tc.tile_pool(name="p1w", bufs=1))
                    p1ev = es1b.enter_context(
                        tc.tile_pool(name="p1ev", bufs=2))
                    p1ps = es1b.enter_context(
                        tc.tile_pool(name="p1ps", bufs=1, space="PSUM"))
                    # group-0-only pools: closed before attention opens so
                    # their PSUM banks and the q/v weights free up
                    es1c = es1b.enter_context(ExitStack())
                    p1wv = es1c.enter_context(
                        tc.tile_pool(name="p1wv", bufs=1))
                    p1vp = es1c.enter_context(
                        tc.tile_pool(name="p1vp", bufs=1, space="PSUM"))
                    p1tp = es1c.enter_context(
                        tc.tile_pool(name="p1tp", bufs=2, space="PSUM"))
                    wts = {}
                    for nm, Wt, pool in (("k", Wk, p1w), ("q", Wq, p1wv),
                                         ("v", Wv, p1wv)):
                        for c in range(NCCH):
                            w = pool.tile([128, W1], MT, tag=f"w{nm}{c}",
                                          name=f"w{nm}{c}")
                            nc.sync.dma_start(
                                out=w, in_=Wt[c * 128:(c + 1) * 128, hgs])
                            wts[nm, c] = w
                    kv_pairs = [(g, min(1024, T - g))
                                for g in range(0, T, 1024)]
                    lts_t = {}

                    def rb_pass(G0, Gsz):
                        ngb = Gsz // 128
                        lts = p1lt.tile([128, NCCH, Gsz], MT, tag="lts",
                                        name="lts")
                        lts_t[G0] = lts
                        for rb in range(ngb):
                            r = G0 + rb * 128
                            rs = slice(rb * 128, (rb + 1) * 128)
                            xt = p1sb.tile([128, C], DT, tag="xt",
                                           name="xt")
                            nc.sync.dma_start(out=xt, in_=xk[r:r + 128, :])
                            ln_m = layernorm(p1sb, p1st, xt, ln1g_t,
                                             ln1b_t, "ln1_gb" in skip)
                            for tq in range(2):
                                tpq = p1tp.tile([128, 4, 128], MT,
                                                tag="tpq", name="tpq")
                                for k in range(4):
                                    c = tq * 4 + k
                                    nc.tensor.transpose(
                                        tpq[:, k, :],
                                        ln_m[:, c * 128:(c + 1) * 128],
                                        identity_m)
                                nc.vector.tensor_copy(
                                    out=lts[:, tq * 4:tq * 4 + 4, rs],
                                    in_=tpq)
                            vps = p1vp.tile([128, 1024], DT, tag="vps",
                                            name="vps")
                            for c in range(NCCH):
                                for half in range(2):
                                    nc.tensor.matmul(
                                        vps[:, half * 512:(half + 1) * 512],
                                        lts[:, c, rs],
                                        wts["v", c][:,
                                                    half * 512:
                                                    (half + 1) * 512],
                                        start=(c == 0),
                                        stop=(c == NCCH - 1))
                            blk = G0 // 128 + rb
                            vpsr = vps.rearrange("p (h d) -> p h d", h=H)
                            if qkv_bias:
                                nc.vector.tensor_tensor(
                                    out=VPA[:, blk, :, 0:64], in0=vpsr,
                                    in1=bvb_t.rearrange(
                                        "p (h d) -> p h d", h=H),
                                    op=OP.add)
                            else:
                                nc.vector.tensor_copy(
                                    out=VPA[:, blk, :, 0:64], in_=vpsr)

                    def k_pair(G0, Gsz, p, with_q):
                        subs = [(s, min(512, Gsz - s))
                                for s in range(0, Gsz, 512)]
                        lts = lts_t[G0]
                        pl = (p - hg * HPG) * 128
                        pls = slice(pl, pl + 128)
                        pss = [p1ps.tile([128, ssz], DT, tag=f"ps{si}",
                                         name=f"ps{si}")
                               for si, (s0, ssz) in enumerate(subs)]
                        for c in range(NCCH):
                            for si, (s0, ssz) in enumerate(subs):
                                nc.tensor.matmul(
                                    pss[si], wts["k", c][:, pls],
                                    lts[:, c, s0:s0 + ssz],
                                    start=(c == 0),
                                    stop=(c == NCCH - 1))
                        for si, (s0, ssz) in enumerate(subs):
                            g0 = G0 + s0
                            kbias = bk_t[:, p:p + 1] \
                                if qkv_bias else 0.0
                            nc.scalar.activation(
                                out=KT[p][:, g0:g0 + ssz], in_=pss[si],
                                func=AF.Identity, bias=kbias)
                        if not with_q:
                            return
                        pss = [p1ps.tile([128, ssz], DT, tag=f"ps{si}",
                                         name=f"ps{si}")
                               for si, (s0, ssz) in enumerate(subs)]
                        for c in range(NCCH):
                            for si, (s0, ssz) in enumerate(subs):
                                nc.tensor.matmul(
                                    pss[si], wts["q", c][:, pls],
                                    lts[:, c, s0:s0 + ssz],
                                    start=(c == 0),
                                    stop=(c == NCCH - 1))
                        for si, (s0, ssz) in enumerate(subs):
                            g0 = G0 + s0
                            qsb = p1ev.tile([128, ssz], MT,
                                            tag="qsb", name="qsb")
                            qbias = bq_t[:, p:p + 1] \
                                if qkv_bias else 0.0
                            nc.scalar.activation(
                                out=qsb, in_=pss[si],
                                func=AF.Identity, bias=qbias)
                            nc.sync.dma_start(
                                out=qT_d[p, :, g0:g0 + ssz],
                                in_=qsb)

                    # phase A: own rows -- LN/V, then K+Q per pair
                    rb_pass(0, 1024)
                    for p in pairs:
                        k_pair(0, 1024, p, with_q=True)
                    # phase B: other-parity rows -- LN/V only; K per pair
                    # is deferred into the attention stream below
                    rb_pass(1024, 1024)
                    es1c.close()

                    # -------- attention, merged with group-1 K ------------
                    with ExitStack() as es2:
                        p2q = es2.enter_context(
                            tc.tile_pool(name="p2q", bufs=3))
                        p2pt = es2.enter_context(
                            tc.tile_pool(name="p2pt", bufs=14))
                        p2st = es2.enter_context(
                            tc.tile_pool(name="p2st", bufs=3))
                        p2sps = es2.enter_context(
                            tc.tile_pool(name="p2sps", bufs=2, space="PSUM"))
                        p2avp = es2.enter_context(
                            tc.tile_pool(name="p2avp", bufs=1, space="PSUM"))
                        p2bc = es2.enter_context(
                            tc.tile_pool(name="p2bc", bufs=1, space="PSUM"))

                        def norm_tail(st):
                            """Deferred per-(p,g) softmax normalize: by now
                            the DVE recip/cast of `st` has completed, so the
                            bc matmul doesn't stall the PE."""
                            avts2, rz16, p_, qs_ = st
                            bcp = p2bc.tile([65, 512], DT, tag="bc",
                                            name="bcp")
                            nc.tensor.matmul(
                                bcp, onesb, rz16, start=True, stop=True)
                            avn = p2st.tile([64, 512], MT, tag="avn",
                                            name="avn")
                            for h2 in range(2):
                                cs = slice(h2 * 256, h2 * 256 + 256)
                                nc.vector.tensor_tensor(
                                    out=avn[:, cs], in0=avts2[h2][0:64, :],
                                    in1=bcp[0:64, cs], op=OP.mult)
                            nc.sync.dma_start(
                                out=avT_d[p_, 0:64, qs_],
                                in_=avn[:, 0:256])
                            nc.sync.dma_start(
                                out=avT_d[p_, 64:128, qs_],
                                in_=avn[:, 256:512])

                        def emit_S(p, g):
                            """S matmuls + exps + masks for one (p, g)
                            segment; returns AV-phase state."""
                            qs = slice(g * 256, (g + 1) * 256)
                            blocks = list(range(2 * g + 2)) + \
                                [NOB + jj for jj in range(2 * g + 2)]
                            nquad = g + 1
                            D_i = 2 * g
                            O_i = 4 * g + 2
                            qt = p2q.tile([128, 256], MT, tag="qt",
                                          name="qt")
                            nc.sync.dma_start(out=qt, in_=qT_d[p, :, qs])
                            pts = {}
                            for qi in range(nquad):
                                quad = blocks[4 * qi:4 * qi + 4]
                                for h2 in range(2):
                                    hs = slice(h2 * 64, h2 * 64 + 64)
                                    sps = p2sps.tile([128, 1024], DT,
                                                     tag="sps", name="sps")
                                    for k, j in enumerate(quad):
                                        ss = slice(k * 256, k * 256 + 256)
                                        nc.tensor.matmul(
                                            sps[:, ss],
                                            KT[p][hs,
                                                  j * 128:(j + 1) * 128],
                                            qt[hs, :], start=True,
                                            stop=True)
                                    pt_sb = p2pt.tile([128, 1024], MT,
                                                      tag="pt", name="pt")
                                    nc.scalar.activation(
                                        out=pt_sb, in_=sps, func=AF.Exp,
                                        scale=0.125)
                                    if 4 * qi <= D_i < 4 * qi + 4:
                                        off = (D_i - 4 * qi) * 256
                                        sl = slice(off, off + 512)
                                        nc.vector.tensor_tensor(
                                            out=pt_sb[:, sl],
                                            in0=pt_sb[:, sl], in1=mD,
                                            op=OP.mult)
                                    if 4 * qi <= O_i < 4 * qi + 4:
                                        off = (O_i - 4 * qi) * 256
                                        sl = slice(off, off + 512)
                                        nc.vector.tensor_tensor(
                                            out=pt_sb[:, sl],
                                            in0=pt_sb[:, sl], in1=mq,
                                            op=OP.mult)
                                    pts[h2, qi] = pt_sb
                            return (pts, blocks, nquad, p, qs, g)

                        def emit_AV(st):
                            """AV matmuls + denominator prep; heads run
                            sequentially through one PSUM accumulator."""
                            pts, blocks, nquad, p, qs, g = st
                            nmm = 4 * g + 4
                            avts2 = {}
                            dent = p2st.tile([1, 512], DT, tag="dent",
                                             name="dent")
                            for h2 in range(2):
                                h = 2 * p + h2
                                avps = p2avp.tile([65, 256], DT,
                                                  tag="avps", name="avps")
                                mi = 0
                                for qi in range(nquad):
                                    quad = blocks[4 * qi:4 * qi + 4]
                                    pt_sb = pts[h2, qi]
                                    for k, j in enumerate(quad):
                                        ss = slice(k * 256, k * 256 + 256)
                                        nc.tensor.matmul(
                                            avps, VPA[:, j, h, :],
                                            pt_sb[:, ss],
                                            start=(mi == 0),
                                            stop=(mi == nmm - 1))
                                        mi += 1
                                avts = p2st.tile([65, 256], DT,
                                                 tag=f"avts{h2}",
                                                 name=f"avts{h2}")
                                nc.vector.tensor_copy(out=avts, in_=avps)
                                # move denominator row to partition 0
                                # (custom DVE ops need base partition 0)
                                cs = slice(h2 * 256, h2 * 256 + 256)
                                nc.sync.dma_start(
                                    out=dent[0:1, cs],
                                    in_=avts[64:65, :])
                                avts2[h2] = avts
                            nc.vector.reciprocal_approx_fast(
                                out=dent, in_=dent)
                            rz16 = p2st.tile([1, 512], MT, tag="rz16",
                                             name="rz16")
                            nc.vector.tensor_copy(out=rz16, in_=dent)
                            return (avts2, rz16, p, qs)

                        # software pipeline: S(k+1) lands before AV(k) so
                        # the scalar engine always has score tiles; each
                        # pair's other-parity K slots in just ahead of its
                        # first segment and hides under the exp stream
                        sched = [(p, g) for p in pairs for g in range(NOG)]
                        av_state = None
                        norm_pending = None
                        for (p, g) in sched:
                            if g == 0:
                                k_pair(1024, 1024, p, with_q=False)
                            for _ in range(2 + g):
                                bft = p2bc.tile([65, 512], DT,
                                                tag="bc", name="bft")
                                nc.tensor.matmul(
                                    bft[0:64, :], identity_m[:, 0:64],
                                    wz, start=True, stop=True)
                            s_next = emit_S(p, g)
                            if norm_pending is not None:
                                norm_tail(norm_pending)
                            if av_state is not None:
                                norm_pending = emit_AV(av_state)
                            av_state = s_next
                        norm_tail(norm_pending)
                        norm_tail(emit_AV(av_state))

            # ===== Phases 3+4: oproj + LN2 + MLP, one pipelined scope ======
            # Emission order keeps the PE dense: oproj -> LN2 rows 0:512 ->
            # fc matmuls for cols 0:512 (LN2 rows 512:1024 run on DVE
            # underneath) -> fc cols 512:1024 -> proj -> output rows.
            ln2T = persist.tile([128, NCCH, OWN], MT, tag="l2t",
                                name="ln2T")
            NFG = (NF + 3) // 4      # fc chunk groups of 4
            with ExitStack() as es3:
                p3av = es3.enter_context(tc.tile_pool(name="p3av", bufs=1))
                p3w = es3.enter_context(tc.tile_pool(name="p3w", bufs=1))
                p3at = es3.enter_context(tc.tile_pool(name="p3at", bufs=1))
                p3sb = es3.enter_context(tc.tile_pool(name="p3sb", bufs=2))
                p3st = es3.enter_context(tc.tile_pool(name="p3st", bufs=8))
                p4h1 = es3.enter_context(tc.tile_pool(name="p4h1", bufs=1))
                p4w = es3.enter_context(tc.tile_pool(name="p4w", bufs=2))
                p4wp = es3.enter_context(tc.tile_pool(name="p4wp", bufs=12))
                p4h2 = es3.enter_context(tc.tile_pool(name="p4h2", bufs=2))
                p3ps = es3.enter_context(
                    tc.tile_pool(name="p3ps", bufs=1, space="PSUM"))
                p4ps = es3.enter_context(
                    tc.tile_pool(name="p4ps", bufs=2, space="PSUM"))
                p3tp = es3.enter_context(
                    tc.tile_pool(name="p3tp", bufs=2, space="PSUM"))
                # PE warm bridge over the avts/wo load window
                for _ in range(8):
                    bps = p3ps.tile([128, 512], DT, tag="ps0", name="bps")
                    nc.tensor.matmul(bps, identity_m, wz, start=True,
                                     stop=True)
                wo_t = []
                for p in range(HP):
                    w = p3w.tile([128, C], MT, tag=f"wo{p}", name=f"wo{p}")
                    nc.sync.dma_start(out=w,
                                      in_=Wo[p * 128:(p + 1) * 128, :])
                    wo_t.append(w)
                avts = [p3av.tile([128, OWN], MT, tag=f"avt{p}",
                                  name=f"avt{p}")
                        for p in range(HP)]
                for p in range(HP):
                    nc.sync.dma_start(out=avts[p], in_=avT_d[p, :, :])
                attnT = [p3at.tile([128, OWN], MT, tag=f"atT{oc}",
                                   name=f"atT{oc}")
                         for oc in range(NCCH)]
                h1T = p4h1.tile([128, NF, OWN], MT, tag="h1",
                                name="h1T")

                def oproj_pass(gi, och):
                    g0, gsz = own_groups[gi]
                    pss = [p3ps.tile([128, gsz], DT, tag=f"ps{j}",
                                     name=f"ps{j}")
                           for j in range(4)]
                    for p in range(HP):
                        for j in range(4):
                            oc = och * 4 + j
                            nc.tensor.matmul(
                                pss[j],
                                wo_t[p][:, oc * 128:(oc + 1) * 128],
                                avts[p][:, g0:g0 + gsz],
                                start=(p == 0), stop=(p == HP - 1))
                    for j in range(4):
                        oc = och * 4 + j
                        obias = bo_t[:, oc:oc + 1] \
                            if bo_t is not None else 0.0
                        nc.scalar.activation(
                            out=attnT[oc][:, g0:g0 + gsz], in_=pss[j],
                            func=AF.Identity, bias=obias)

                def ln2_rows(rb):
                    r = rb * 128
                    xo = p3sb.tile([128, C], DT, tag="xo", name="xo")
                    nc.sync.dma_start(out=xo, in_=xk[r:r + 128, :])
                    x2 = p3sb.tile([128, C], DT, tag="x2", name="x2")
                    for oc in range(NCCH):
                        tp = p3tp.tile([128, 128], MT, tag="tp", name="tp")
                        nc.tensor.transpose(
                            tp, attnT[oc][:, rb * 128:(rb + 1) * 128],
                            identity_m)
                        nc.vector.tensor_tensor(
                            out=x2[:, oc * 128:(oc + 1) * 128], in0=tp,
                            in1=xo[:, oc * 128:(oc + 1) * 128],
                            op=OP.add)
                    nc.sync.dma_start(out=x2_d[r:r + 128, :], in_=x2)
                    ln_m = layernorm(p3sb, p3st, x2, ln2g_t, ln2b_t,
                                     "ln2_gb" in skip)
                    for c in range(NCCH):
                        tp = p3tp.tile([128, 128], MT, tag="tp",
                                       name="tpm")
                        nc.tensor.transpose(
                            tp, ln_m[:, c * 128:(c + 1) * 128],
                            identity_m)
                        nc.vector.tensor_copy(
                            out=ln2T[:, c, r:r + 128], in_=tp)

                def fc_pass(gi, inject=()):
                    g0, gsz = own_groups[gi]
                    for fcg in range(NFG):
                        nfl = min(4, NF - fcg * 4)
                        wfs = []
                        for c in range(NCCH):
                            w = p4w.tile([128, 512], MT, tag=f"wf{c}",
                                         name=f"wf{c}")
                            nc.sync.dma_start(
                                out=w[:, 0:128 * nfl],
                                in_=W_fc[c * 128:(c + 1) * 128,
                                         fcg * 512:fcg * 512 + 128 * nfl])
                            wfs.append(w)
                        for fl in range(nfl):
                            fc = fcg * 4 + fl
                            fls = slice(fl * 128, (fl + 1) * 128)
                            ps = p4ps.tile([128, gsz], DT, tag="fps",
                                           name="fps")
                            for c in range(NCCH):
                                nc.tensor.matmul(
                                    ps, wfs[c][:, fls],
                                    ln2T[:, c, g0:g0 + gsz],
                                    start=(c == 0), stop=(c == NCCH - 1))
                            gbias = bfc_t[:, fc:fc + 1] \
                                if bfc_t is not None else 0.0
                            nc.scalar.activation(
                                out=h1T[:, fc, g0:g0 + gsz], in_=ps,
                                func=AF.Gelu_apprx_tanh, bias=gbias)
                        if fcg in inject:
                            ln2_rows(inject[fcg])

                # interleave: oproj passes hide LN2 of rows 0:512; fc
                # weight-groups hide LN2 of rows 512:1024
                oproj_pass(0, 0)
                oproj_pass(0, 1)
                ln2_rows(0)
                oproj_pass(1, 0)
                ln2_rows(1)
                oproj_pass(1, 1)
                ln2_rows(2)
                ln2_rows(3)
                fc_pass(0, inject={0: 4, 1: 5, 2: 6, 3: 7})
                fc_pass(1)

                NOC2 = (NCCH + 1) // 2
                for ocp in range(NOC2):
                    nol = min(2, NCCH - ocp * 2)
                    pss = {}
                    for ol in range(nol):
                        for gi in range(len(own_groups)):
                            pss[ol, gi] = p3ps.tile(
                                [128, own_groups[gi][1]], DT,
                                tag=f"ps{ol * 2 + gi}",
                                name=f"ps{ol * 2 + gi}")
                    for c2 in range(NF):
                        w = p4wp.tile([128, 256], MT, tag="wp", name="wp")
                        nc.sync.dma_start(
                            out=w[:, 0:128 * nol],
                            in_=W_proj[c2 * 128:(c2 + 1) * 128,
                                       ocp * 256:ocp * 256 + 128 * nol])
                        for ol in range(nol):
                            for gi, (g0, gsz) in enumerate(own_groups):
                                nc.tensor.matmul(
                                    pss[ol, gi],
                                    w[:, ol * 128:(ol + 1) * 128],
                                    h1T[:, c2, g0:g0 + gsz],
                                    start=(c2 == 0), stop=(c2 == NF - 1))
                    h2s = {}
                    for ol in range(nol):
                        oc = ocp * 2 + ol
                        h2s[ol] = p4h2.tile([128, OWN], MT, tag=f"h2_{ol}",
                                            name=f"h2_{ol}")
                        for gi, (g0, gsz) in enumerate(own_groups):
                            pbias = bpr_t[:, oc:oc + 1] \
                                if bpr_t is not None else 0.0
                            nc.scalar.activation(
                                out=h2s[ol][:, g0:g0 + gsz],
                                in_=pss[ol, gi],
                                func=AF.Identity, bias=pbias)
                    # residual + transpose back, column strip of this ocp;
                    # overlaps the next ocp's proj matmuls on the PE
                    cw = 128 * nol
                    for rb in range(OWN // 128):
                        r = rb * 128
                        x2t = p3sb.tile([128, 256], DT, tag="x2t",
                                        name="x2t")
                        nc.sync.dma_start(
                            out=x2t[:, 0:cw],
                            in_=x2_d[r:r + 128,
                                     ocp * 256:ocp * 256 + cw])
                        outt = p3sb.tile([128, 256], DT, tag="outt",
                                         name="outt")
                        for ol in range(nol):
                            tp = p3tp.tile([128, 128], MT, tag="tp",
                                           name="tp")
                            nc.tensor.transpose(
                                tp, h2s[ol][:, rb * 128:(rb + 1) * 128],
                                identity_m)
                            nc.vector.tensor_tensor(
                                out=outt[:, ol * 128:(ol + 1) * 128],
                                in0=tp,
                                in1=x2t[:, ol * 128:(ol + 1) * 128],
                                op=OP.add)
                        nc.sync.dma_start(
                            out=out[r:r + 128,
                                    ocp * 256:ocp * 256 + cw],
                            in_=outt[:, 0:cw])

    nc.compile()
    return nc


# ---------------------------------------------------------------------------
# host-side sharding
# ---------------------------------------------------------------------------

def detect_skips(inputs):
    def z(*ks):
        return all(not np.asarray(inputs[k]).any() for k in ks)
    skips = []
    if z("bq", "bk", "bv"):
        skips.append("qkv_bias")
    if z("bo"):
        skips.append("o_bias")
    if z("b_fc"):
        skips.append("fc_bias")
    if z("b_proj"):
        skips.append("proj_bias")
    if np.all(np.asarray(inputs["ln1_g"]) == 1.0) and z("ln1_b"):
        skips.append("ln1_gb")
    if np.all(np.asarray(inputs["ln2_g"]) == 1.0) and z("ln2_b"):
        skips.append("ln2_gb")
    return tuple(skips)


def shard_inputs(inputs, T=2048, C=1024, n_batch=4, mm_dtype="bf16"):
    """Build per-core in_maps for the 8-core SPMD launch."""
    import ml_dtypes
    wdt = ml_dtypes.bfloat16 if mm_dtype == "bf16" else np.float32
    f8 = ml_dtypes.float8_e4m3
    NB = T // 128
    NOB = NB // 2
    x = np.asarray(inputs["x"], np.float32)
    shared = {}
    for k in ("Wq", "Wk", "Wv", "Wo", "bq", "bk", "bv", "bo",
              "ln1_g", "ln1_b", "ln2_g", "ln2_b",
              "W_fc", "b_fc", "W_proj", "b_proj"):
        arr = np.asarray(inputs[k], np.float32)
        if k[0] == "W":
            arr = arr.astype(wdt)
        shared[k] = np.ascontiguousarray(arr)
    in_maps = []
    for b in range(n_batch):
        xb = x[b].reshape(NB, 128, C)
        for h in range(2):
            perm = [2 * j + h for j in range(NOB)] + \
                   [2 * j + (1 - h) for j in range(NOB)]
            xkp = np.ascontiguousarray(xb[perm].reshape(T, C))
            # multiplicative 0/1 parity masks for kv-blocks NOB+2g (slot 0)
            # and NOB+2g+1 (slot 1)
            mqa = np.ones((128, 2, 256), np.float32)
            if h == 0:
                mqa[:, 0, 0:128] = 0.0
                mqa[:, 1, :] = 0.0
            else:
                mqa[:, 1, 0:128] = 0.0
            m = dict(shared)
            m["xk"] = xkp
            m["maskq"] = mqa.astype(wdt)
            in_maps.append(m)
    return in_maps


def unshard_output(results, T=2048, C=1024, n_batch=4):
    NB = T // 128
    NOB = NB // 2
    out = np.empty((n_batch, T, C), np.float32)
    ci = 0
    for b in range(n_batch):
        for h in range(2):
            o = results[ci]["out"].reshape(NOB, 128, C)
            for i in range(NOB):
                g = 2 * i + h
                out[b, g * 128:(g + 1) * 128, :] = o[i]
            ci += 1
    return out


_CACHE = {}
_LOCK = threading.Lock()


def _get_program(T, C, H, skip):
    key = (T, C, H, skip)
    with _LOCK:
        if key not in _CACHE:
            _CACHE[key] = build_block_program(T=T, C=C, H=H, skip=skip)
        return _CACHE[key]


def run(inputs, trace=False, **kw):
    x = np.asarray(inputs["x"])
    B, T, C = x.shape
    H = 16
    skip = detect_skips(inputs)
    nc = _get_program(T, C, H, skip)
    in_maps = shard_inputs(inputs, T=T, C=C, n_batch=B)
    res = bass_utils.run_bass_kernel_spmd(
        nc, in_maps, core_ids=list(range(8)), trace=trace, **kw)
    return unshard_output(res.results, T=T, C=C, n_batch=B), res


def kernel(**inputs):
    return run(inputs)[0]



# revision 32
# speedup vs baseline: 1.0070x; 1.0070x over previous
"""Trainium2 Bass kernel for a dense transformer block (LN->attn->LN->MLP).

Sharding: 8 cores = (batch b in 0..3, parity h in 0..1). Core (b,h) owns the
interleaved 128-row q-blocks {h, h+2, ...} of batch b.  Host permutes the
batch's rows so the core's own blocks come first; causal structure is then
identical on every core (uniform SPMD program): own q-block i attends to
permuted kv-blocks [0..i] (own parity, triangular mask on block i) and
[NOB..NOB+i] (other parity; parity-dependent masking supplied as per-core
mask data).

Attention (v3): q-blocks processed in groups of 256 columns; S^T[kv,q] tiles
for two kv-blocks share one PSUM bank and one ACT exp; AV computes
av'^T = V'^T @ P^T (lhsT = V' with appended ones column -> softmax sums in
row 64), then a short transpose chain normalizes and re-transposes per
128-q-half.  All matmuls are bf16 (fp32 matmul runs 2-pass LOW_HIGH);
accumulation stays fp32 in PSUM, LN/softmax/residual arithmetic fp32.
"""

import math
import threading
from contextlib import ExitStack

import numpy as np

import concourse.bass as bass
import concourse.mybir as mybir
import concourse.tile as tile
from concourse import bacc, bass_utils
from concourse.masks import (make_identity, make_lower_triangular,
                             make_upper_triangular)

AF = mybir.ActivationFunctionType
OP = mybir.AluOpType
DT = mybir.dt.float32
BF = mybir.dt.bfloat16
F8 = mybir.dt.float8e4
PM = mybir.MatmulPerfMode
W8_SCALE = 64.0

LN_EPS = 1e-5
MASK_VAL = -30000.0



def build_block_program(T=2048, C=1024, H=16, gelu_mode="hw",
                        mm_dtype="bf16", skip=()):
    """Build the per-core SPMD Bass program. Returns compiled Bacc.

    skip: subset of {"qkv_bias","o_bias","fc_bias","proj_bias","ln1_gb",
    "ln2_gb"} -- ops elided because the host verified the params are
    identity (zero bias / unit gain).
    """
    D = 64
    GELU_C = math.sqrt(2.0 / math.pi)
    MT = BF if mm_dtype == "bf16" else DT
    NB = T // 128            # kv blocks (permuted)
    NOB = NB // 2            # own q-blocks
    NOG = NOB // 2           # own q-groups (256 rows)
    OWN = NOB * 128          # own rows
    NCCH = C // 128          # feature chunks
    F = 4 * C
    NF = F // 128
    HP = H // 2              # head pairs
    HPG = HP                 # single pass: all head pairs resident
    W1 = HPG * 128           # qkv weight tile width
    BN_W = min(C, 512)       # bn_stats subgroup width
    NST = C // BN_W

    GSZ = min(512, OWN)
    kv_groups = [(g, min(512, T - g)) for g in range(0, T, 512)]
    own_groups = [(g, min(GSZ, OWN - g)) for g in range(0, OWN, GSZ)]

    nc = bacc.Bacc("TRN2", target_bir_lowering=False, debug=False)

    xk = nc.dram_tensor("xk", [T, C], DT, kind="ExternalInput")
    maskq = nc.dram_tensor("maskq", [128, 2, 256], MT, kind="ExternalInput")
    Wq = nc.dram_tensor("Wq", [C, C], MT, kind="ExternalInput")
    Wk = nc.dram_tensor("Wk", [C, C], MT, kind="ExternalInput")
    Wv = nc.dram_tensor("Wv", [C, C], MT, kind="ExternalInput")
    Wo = nc.dram_tensor("Wo", [C, C], MT, kind="ExternalInput")
    bq = nc.dram_tensor("bq", [C], DT, kind="ExternalInput")
    bk = nc.dram_tensor("bk", [C], DT, kind="ExternalInput")
    bv = nc.dram_tensor("bv", [C], DT, kind="ExternalInput")
    bo = nc.dram_tensor("bo", [C], DT, kind="ExternalInput")
    ln1_g = nc.dram_tensor("ln1_g", [C], DT, kind="ExternalInput")
    ln1_b = nc.dram_tensor("ln1_b", [C], DT, kind="ExternalInput")
    ln2_g = nc.dram_tensor("ln2_g", [C], DT, kind="ExternalInput")
    ln2_b = nc.dram_tensor("ln2_b", [C], DT, kind="ExternalInput")
    W_fc = nc.dram_tensor("W_fc", [C, F], MT, kind="ExternalInput")
    b_fc = nc.dram_tensor("b_fc", [F], DT, kind="ExternalInput")
    W_proj = nc.dram_tensor("W_proj", [F, C], MT, kind="ExternalInput")
    b_proj = nc.dram_tensor("b_proj", [C], DT, kind="ExternalInput")
    out = nc.dram_tensor("out", [OWN, C], DT, kind="ExternalOutput")

    with tile.TileContext(nc) as tc:
        with ExitStack() as es0:
            consts = es0.enter_context(tc.tile_pool(name="consts", bufs=1))
            persist = es0.enter_context(tc.tile_pool(name="persist", bufs=1))
            dram = es0.enter_context(
                tc.tile_pool(name="dram", bufs=1, space="DRAM"))
            identity_m = consts.tile([128, 128], MT)
            make_identity(nc, identity_m)
            identity = consts.tile([128, 128], DT)
            make_identity(nc, identity)
            # multiplicative diag mask for own-parity block pair (2g, 2g+1):
            # [triu1 | ones | zeros | triu1] over S^T tiles [kv, q]
            mD = consts.tile([128, 512], MT)
            make_upper_triangular(nc, mD[:, 0:128], val=1.0, diag=True)
            nc.gpsimd.memset(mD[:, 128:256], 1.0)
            nc.gpsimd.memset(mD[:, 256:384], 0.0)
            make_upper_triangular(nc, mD[:, 384:512], val=1.0, diag=True)
            # per-core multiplicative parity masks (0/1), [slot0|slot1]
            mq = consts.tile([128, 512], MT)
            nc.gpsimd.dma_start(out=mq, in_=maskq[:, :, :])
            eps_t = consts.tile([128, 1], DT)
            nc.vector.memset(eps_t, LN_EPS)
            # bf16 ones row (softmax-denominator outer-product broadcast)
            onesb = consts.tile([1, 65], MT)
            nc.vector.memset(onesb, 1.0)
            # PE warmup: keep the systolic array busy through the initial
            # DMA window so the HAM clock gate opens before real matmuls
            wz = consts.tile([128, 512], MT)
            nc.gpsimd.memset(wz, 0.0)
            with ExitStack() as eswu:
                wup = eswu.enter_context(
                    tc.tile_pool(name="wup", bufs=1, space="PSUM"))
                for _ in range(40):
                    wps = wup.tile([128, 512], DT, tag="wps", name="wps")
                    nc.tensor.matmul(wps, identity_m, wz, start=True,
                                     stop=True)

            def bcast_tile(vec):
                t = consts.tile([128, C], DT, tag=f"bc_{vec.name}",
                                name=f"bc_{vec.name}")
                src = bass.AP(tensor=vec, offset=0, ap=[[0, 128], [1, C]])
                nc.gpsimd.dma_start(out=t, in_=src)
                return t

            ln1g_t = bcast_tile(ln1_g) if "ln1_gb" not in skip else None
            ln1b_t = bcast_tile(ln1_b) if "ln1_gb" not in skip else None
            ln2g_t = bcast_tile(ln2_g) if "ln2_gb" not in skip else None
            ln2b_t = bcast_tile(ln2_b) if "ln2_gb" not in skip else None

            def chunk_tile(vec, n):
                t = consts.tile([128, n], DT, tag=f"ck_{vec.name}",
                                name=f"ck_{vec.name}")
                nc.gpsimd.dma_start(
                    out=t, in_=vec.ap().rearrange("(a p) -> p a", p=128))
                return t

            qkv_bias = "qkv_bias" not in skip
            bq_t = chunk_tile(bq, NCCH) if qkv_bias else None
            bvb_t = bcast_tile(bv) if qkv_bias else None
            bk_t = chunk_tile(bk, NCCH) if qkv_bias else None
            bv_t = chunk_tile(bv, NCCH) if qkv_bias else None
            bo_t = chunk_tile(bo, NCCH) if "o_bias" not in skip else None
            bfc_t = chunk_tile(b_fc, NF) if "fc_bias" not in skip else None
            bpr_t = chunk_tile(b_proj, NCCH) \
                if "proj_bias" not in skip else None

            qT_d = dram.tile([HP, 128, OWN], MT, tag="qT", name="qT_d")
            avT_d = dram.tile([HP, 128, OWN], MT, tag="avT", name="avT_d")
            x2_d = dram.tile([OWN, C], DT, tag="x2", name="x2_d")

            def layernorm(pool, spool, xt, g_t, b_t, skip_gb):
                """LN of xt [128,C] f32 -> new MT tile."""
                stats = spool.tile([128, NST, 6], DT, tag="stats",
                                   name="stats")
                mv = spool.tile([128, 2], DT, tag="mv", name="mv")
                for s in range(NST):
                    nc.vector.bn_stats(out=stats[:, s, :],
                                       in_=xt[:, s * BN_W:(s + 1) * BN_W])
                nc.vector.bn_aggr(out=mv, in_=stats)
                rstd = spool.tile([128, 1], DT, tag="rstd", name="rstd")
                nc.scalar.activation(out=rstd, in_=mv[:, 1:2],
                                     func=AF.Sqrt, bias=eps_t[:, :])
                nc.vector.reciprocal(out=rstd, in_=rstd)
                ln_m = pool.tile([128, C], MT, tag="ln_m", name="ln_m")
                if skip_gb:
                    nc.vector.tensor_scalar(
                        out=ln_m, in0=xt, scalar1=mv[:, 0:1], scalar2=rstd,
                        op0=OP.subtract, op1=OP.mult)
                else:
                    ln = pool.tile([128, C], DT, tag="ln", name="ln")
                    nc.vector.tensor_scalar(
                        out=ln, in0=xt, scalar1=mv[:, 0:1], scalar2=rstd,
                        op0=OP.subtract, op1=OP.mult)
                    nc.vector.tensor_tensor(out=ln, in0=ln, in1=g_t,
                                            op=OP.mult)
                    nc.vector.tensor_tensor(out=ln_m, in0=ln, in1=b_t,
                                            op=OP.add)
                return ln_m

            # ===== Phases 1+2 per head-group =====
            for hg in range(HP // HPG):
                pairs = list(range(hg * HPG, (hg + 1) * HPG))
                hgs = slice(hg * W1, (hg + 1) * W1)
                with ExitStack() as es1:
                    kvp = es1.enter_context(tc.tile_pool(name="kvp", bufs=1))
                    KT = {p: kvp.tile([128, T], MT, tag=f"kt{p}",
                                      name=f"kt{p}") for p in pairs}
                    # V in [kv, head, dim|ones] layout, built directly by
                    # kv-block-major matmuls (lnT stationary, Wv moving) --
                    # no per-head transposes
                    VPA = kvp.tile([128, NB, H, 65], MT, tag="vpa",
                                   name="vpa")
                    nc.gpsimd.memset(VPA[:, :, :, 64:65], 1.0)
                    es1b = es1.enter_context(ExitStack())
                    p1sb = es1b.enter_context(
                        tc.tile_pool(name="p1sb", bufs=2))
                    p1st = es1b.enter_context(
                        tc.tile_pool(name="p1st", bufs=8))
                    p1lt = es1b.enter_context(
                        tc.tile_pool(name="p1lt", bufs=2))
                    p1w = es1b.enter_context(
                        tc.tile_pool(name="p1w", bufs=1))
                    p1ev = es1b.enter_context(
                        tc.tile_pool(name="p1ev", bufs=2))
                    # group-0-only pools: closed before attention opens so
                    # their PSUM banks and the q/v weights free up
                    es1c = es1b.enter_context(ExitStack())
                    p1ps = es1c.enter_context(
                        tc.tile_pool(name="p1ps", bufs=1, space="PSUM"))
                    p1wv = es1c.enter_context(
                        tc.tile_pool(name="p1wv", bufs=1))
                    p1vp = es1c.enter_context(
                        tc.tile_pool(name="p1vp", bufs=1, space="PSUM"))
                    p1tp = es1c.enter_context(
                        tc.tile_pool(name="p1tp", bufs=2, space="PSUM"))
                    wts = {}
                    for nm, Wt, pool in (("k", Wk, p1w), ("q", Wq, p1wv),
                                         ("v", Wv, p1wv)):
                        for c in range(NCCH):
                            w = pool.tile([128, W1], MT, tag=f"w{nm}{c}",
                                          name=f"w{nm}{c}")
                            nc.sync.dma_start(
                                out=w, in_=Wt[c * 128:(c + 1) * 128, hgs])
                            wts[nm, c] = w
                    kv_pairs = [(g, min(1024, T - g))
                                for g in range(0, T, 1024)]
                    lts_t = {}

                    def rb_pass(G0, Gsz):
                        ngb = Gsz // 128
                        lts = p1lt.tile([128, NCCH, Gsz], MT, tag="lts",
                                        name="lts")
                        lts_t[G0] = lts
                        for rb in range(ngb):
                            r = G0 + rb * 128
                            rs = slice(rb * 128, (rb + 1) * 128)
                            xt = p1sb.tile([128, C], DT, tag="xt",
                                           name="xt")
                            nc.sync.dma_start(out=xt, in_=xk[r:r + 128, :])
                            ln_m = layernorm(p1sb, p1st, xt, ln1g_t,
                                             ln1b_t, "ln1_gb" in skip)
                            for tq in range(2):
                                tpq = p1tp.tile([128, 4, 128], MT,
                                                tag="tpq", name="tpq")
                                for k in range(4):
                                    c = tq * 4 + k
                                    nc.tensor.transpose(
                                        tpq[:, k, :],
                                        ln_m[:, c * 128:(c + 1) * 128],
                                        identity_m)
                                nc.vector.tensor_copy(
                                    out=lts[:, tq * 4:tq * 4 + 4, rs],
                                    in_=tpq)
                            vps = p1vp.tile([128, 1024], DT, tag="vps",
                                            name="vps")
                            for c in range(NCCH):
                                for half in range(2):
                                    nc.tensor.matmul(
                                        vps[:, half * 512:(half + 1) * 512],
                                        lts[:, c, rs],
                                        wts["v", c][:,
                                                    half * 512:
                                                    (half + 1) * 512],
                                        start=(c == 0),
                                        stop=(c == NCCH - 1))
                            blk = G0 // 128 + rb
                            vpsr = vps.rearrange("p (h d) -> p h d", h=H)
                            if qkv_bias:
                                nc.vector.tensor_tensor(
                                    out=VPA[:, blk, :, 0:64], in0=vpsr,
                                    in1=bvb_t.rearrange(
                                        "p (h d) -> p h d", h=H),
                                    op=OP.add)
                            else:
                                nc.vector.tensor_copy(
                                    out=VPA[:, blk, :, 0:64], in_=vpsr)

                    def k_pair(G0, Gsz, p, with_q, kalloc=None):
                        subs = [(s, min(512, Gsz - s))
                                for s in range(0, Gsz, 512)]
                        lts = lts_t[G0]
                        pl = (p - hg * HPG) * 128
                        pls = slice(pl, pl + 128)
                        if kalloc is None:
                            def kalloc(si):
                                return p1ps.tile([128, 512], DT,
                                                 tag=f"ps{si}",
                                                 name=f"ps{si}")
                        for si, (s0, ssz) in enumerate(subs):
                            ps = kalloc(si)
                            for c in range(NCCH):
                                nc.tensor.matmul(
                                    ps, wts["k", c][:, pls],
                                    lts[:, c, s0:s0 + ssz],
                                    start=(c == 0),
                                    stop=(c == NCCH - 1))
                            g0 = G0 + s0
                            kbias = bk_t[:, p:p + 1] \
                                if qkv_bias else 0.0
                            nc.scalar.activation(
                                out=KT[p][:, g0:g0 + ssz], in_=ps,
                                func=AF.Identity, bias=kbias)
                        if not with_q:
                            return
                        pss = [p1ps.tile([128, ssz], DT, tag=f"ps{si}",
                                         name=f"ps{si}")
                               for si, (s0, ssz) in enumerate(subs)]
                        for c in range(NCCH):
                            for si, (s0, ssz) in enumerate(subs):
                                nc.tensor.matmul(
                                    pss[si], wts["q", c][:, pls],
                                    lts[:, c, s0:s0 + ssz],
                                    start=(c == 0),
                                    stop=(c == NCCH - 1))
                        for si, (s0, ssz) in enumerate(subs):
                            g0 = G0 + s0
                            qsb = p1ev.tile([128, ssz], MT,
                                            tag="qsb", name="qsb")
                            qbias = bq_t[:, p:p + 1] \
                                if qkv_bias else 0.0
                            nc.scalar.activation(
                                out=qsb, in_=pss[si],
                                func=AF.Identity, bias=qbias)
                            nc.sync.dma_start(
                                out=qT_d[p, :, g0:g0 + ssz],
                                in_=qsb)

                    # phase A: own rows -- LN/V, then K+Q per pair
                    rb_pass(0, 1024)
                    for p in pairs:
                        k_pair(0, 1024, p, with_q=True)
                    # phase B: other-parity rows -- LN/V only; K per pair
                    # is deferred into the attention stream below
                    rb_pass(1024, 1024)
                    es1c.close()

                    # -------- attention, merged with group-1 K ------------
                    with ExitStack() as es2:
                        p2q = es2.enter_context(
                            tc.tile_pool(name="p2q", bufs=3))
                        p2pt = es2.enter_context(
                            tc.tile_pool(name="p2pt", bufs=14))
                        p2st = es2.enter_context(
                            tc.tile_pool(name="p2st", bufs=3))
                        p2sps = es2.enter_context(
                            tc.tile_pool(name="p2sps", bufs=2, space="PSUM"))
                        p2avp = es2.enter_context(
                            tc.tile_pool(name="p2avp", bufs=1, space="PSUM"))
                        p2bc = es2.enter_context(
                            tc.tile_pool(name="p2bc", bufs=1, space="PSUM"))

                        def norm_tail(st):
                            """Deferred per-(p,g) softmax normalize: by now
                            the DVE recip/cast of `st` has completed, so the
                            bc matmul doesn't stall the PE."""
                            avts2, rz16, p_, qs_ = st
                            bcp = p2bc.tile([65, 512], DT, tag="bc",
                                            name="bcp")
                            nc.tensor.matmul(
                                bcp, onesb, rz16, start=True, stop=True)
                            avn = p2st.tile([64, 512], MT, tag="avn",
                                            name="avn")
                            for h2 in range(2):
                                cs = slice(h2 * 256, h2 * 256 + 256)
                                nc.vector.tensor_tensor(
                                    out=avn[:, cs], in0=avts2[h2][0:64, :],
                                    in1=bcp[0:64, cs], op=OP.mult)
                            nc.sync.dma_start(
                                out=avT_d[p_, 0:64, qs_],
                                in_=avn[:, 0:256])
                            nc.sync.dma_start(
                                out=avT_d[p_, 64:128, qs_],
                                in_=avn[:, 256:512])

                        def emit_S(p, g):
                            """S matmuls + exps + masks for one (p, g)
                            segment; returns AV-phase state."""
                            qs = slice(g * 256, (g + 1) * 256)
                            blocks = list(range(2 * g + 2)) + \
                                [NOB + jj for jj in range(2 * g + 2)]
                            nquad = g + 1
                            D_i = 2 * g
                            O_i = 4 * g + 2
                            qt = p2q.tile([128, 256], MT, tag="qt",
                                          name="qt")
                            nc.sync.dma_start(out=qt, in_=qT_d[p, :, qs])
                            pts = {}
                            for qi in range(nquad):
                                quad = blocks[4 * qi:4 * qi + 4]
                                for h2 in range(2):
                                    hs = slice(h2 * 64, h2 * 64 + 64)
                                    sps = p2sps.tile([128, 1024], DT,
                                                     tag="sps", name="sps")
                                    for k, j in enumerate(quad):
                                        ss = slice(k * 256, k * 256 + 256)
                                        nc.tensor.matmul(
                                            sps[:, ss],
                                            KT[p][hs,
                                                  j * 128:(j + 1) * 128],
                                            qt[hs, :], start=True,
                                            stop=True)
                                    pt_sb = p2pt.tile([128, 1024], MT,
                                                      tag="pt", name="pt")
                                    nc.scalar.activation(
                                        out=pt_sb, in_=sps, func=AF.Exp,
                                        scale=0.125)
                                    if 4 * qi <= D_i < 4 * qi + 4:
                                        off = (D_i - 4 * qi) * 256
                                        sl = slice(off, off + 512)
                                        nc.vector.tensor_tensor(
                                            out=pt_sb[:, sl],
                                            in0=pt_sb[:, sl], in1=mD,
                                            op=OP.mult)
                                    if 4 * qi <= O_i < 4 * qi + 4:
                                        off = (O_i - 4 * qi) * 256
                                        sl = slice(off, off + 512)
                                        nc.vector.tensor_tensor(
                                            out=pt_sb[:, sl],
                                            in0=pt_sb[:, sl], in1=mq,
                                            op=OP.mult)
                                    pts[h2, qi] = pt_sb
                            return (pts, blocks, nquad, p, qs, g)

                        def emit_AV(st):
                            """AV matmuls + denominator prep; heads run
                            sequentially through one PSUM accumulator."""
                            pts, blocks, nquad, p, qs, g = st
                            nmm = 4 * g + 4
                            avts2 = {}
                            dent = p2st.tile([1, 512], DT, tag="dent",
                                             name="dent")
                            for h2 in range(2):
                                h = 2 * p + h2
                                avps = p2avp.tile([65, 256], DT,
                                                  tag=f"avps{h2}",
                                                  name=f"avps{h2}")
                                mi = 0
                                for qi in range(nquad):
                                    quad = blocks[4 * qi:4 * qi + 4]
                                    pt_sb = pts[h2, qi]
                                    for k, j in enumerate(quad):
                                        ss = slice(k * 256, k * 256 + 256)
                                        nc.tensor.matmul(
                                            avps, VPA[:, j, h, :],
                                            pt_sb[:, ss],
                                            start=(mi == 0),
                                            stop=(mi == nmm - 1))
                                        mi += 1
                                avts = p2st.tile([65, 256], DT,
                                                 tag=f"avts{h2}",
                                                 name=f"avts{h2}")
                                nc.vector.tensor_copy(out=avts, in_=avps)
                                # move denominator row to partition 0
                                # (custom DVE ops need base partition 0)
                                cs = slice(h2 * 256, h2 * 256 + 256)
                                nc.sync.dma_start(
                                    out=dent[0:1, cs],
                                    in_=avts[64:65, :])
                                avts2[h2] = avts
                            nc.vector.reciprocal_approx_fast(
                                out=dent, in_=dent)
                            rz16 = p2st.tile([1, 512], MT, tag="rz16",
                                             name="rz16")
                            nc.vector.tensor_copy(out=rz16, in_=dent)
                            return (avts2, rz16, p, qs)

                        # software pipeline: S(k+1) lands before AV(k) so
                        # the scalar engine always has score tiles; each
                        # pair's other-parity K slots in just ahead of its
                        # first segment and hides under the exp stream
                        def kalloc_bc(si):
                            return p2bc.tile([128, 512], DT, tag="bc",
                                             name="kps")

                        sched = [(p, g) for p in pairs for g in range(NOG)]
                        k_pair(1024, 1024, pairs[0], with_q=False,
                               kalloc=kalloc_bc)
                        av_state = None
                        norm_pending = None
                        for si_, (p, g) in enumerate(sched):
                            if g == 2 and p != pairs[-1]:
                                k_pair(1024, 1024, p + 1, with_q=False,
                                       kalloc=kalloc_bc)
                            for _ in range(1 + g if g else 0):
                                bft = p2bc.tile([65, 512], DT,
                                                tag="bc", name="bft")
                                nc.tensor.matmul(
                                    bft[0:64, :], identity_m[:, 0:64],
                                    wz, start=True, stop=True)
                            s_next = emit_S(p, g)
                            if norm_pending is not None:
                                norm_tail(norm_pending)
                            if av_state is not None:
                                norm_pending = emit_AV(av_state)
                            av_state = s_next
                        norm_tail(norm_pending)
                        norm_tail(emit_AV(av_state))

            # ===== Phases 3+4: oproj + LN2 + MLP, one pipelined scope ======
            # Emission order keeps the PE dense: oproj -> LN2 rows 0:512 ->
            # fc matmuls for cols 0:512 (LN2 rows 512:1024 run on DVE
            # underneath) -> fc cols 512:1024 -> proj -> output rows.
            ln2T = persist.tile([128, NCCH, OWN], MT, tag="l2t",
                                name="ln2T")
            NFG = (NF + 3) // 4      # fc chunk groups of 4
            with ExitStack() as es3:
                p3av = es3.enter_context(tc.tile_pool(name="p3av", bufs=1))
                p3w = es3.enter_context(tc.tile_pool(name="p3w", bufs=1))
                p3at = es3.enter_context(tc.tile_pool(name="p3at", bufs=1))
                p3sb = es3.enter_context(tc.tile_pool(name="p3sb", bufs=2))
                p3st = es3.enter_context(tc.tile_pool(name="p3st", bufs=8))
                p4h1 = es3.enter_context(tc.tile_pool(name="p4h1", bufs=1))
                p4w = es3.enter_context(tc.tile_pool(name="p4w", bufs=2))
                p4wp = es3.enter_context(tc.tile_pool(name="p4wp", bufs=12))
                p4h2 = es3.enter_context(tc.tile_pool(name="p4h2", bufs=2))
                p3ps = es3.enter_context(
                    tc.tile_pool(name="p3ps", bufs=1, space="PSUM"))
                p4ps = es3.enter_context(
                    tc.tile_pool(name="p4ps", bufs=2, space="PSUM"))
                p3tp = es3.enter_context(
                    tc.tile_pool(name="p3tp", bufs=2, space="PSUM"))
                # PE warm bridge over the avts/wo load window
                for _ in range(8):
                    bps = p3ps.tile([128, 512], DT, tag="ps0", name="bps")
                    nc.tensor.matmul(bps, identity_m, wz, start=True,
                                     stop=True)
                wo_t = []
                for p in range(HP):
                    w = p3w.tile([128, C], MT, tag=f"wo{p}", name=f"wo{p}")
                    nc.sync.dma_start(out=w,
                                      in_=Wo[p * 128:(p + 1) * 128, :])
                    wo_t.append(w)
                avts = [p3av.tile([128, OWN], MT, tag=f"avt{p}",
                                  name=f"avt{p}")
                        for p in range(HP)]
                for p in range(HP):
                    nc.sync.dma_start(out=avts[p], in_=avT_d[p, :, :])
                attnT = [p3at.tile([128, OWN], MT, tag=f"atT{oc}",
                                   name=f"atT{oc}")
                         for oc in range(NCCH)]
                h1T = p4h1.tile([128, NF, OWN], MT, tag="h1",
                                name="h1T")

                def oproj_pass(gi, och):
                    g0, gsz = own_groups[gi]
                    pss = [p3ps.tile([128, gsz], DT, tag=f"ps{j}",
                                     name=f"ps{j}")
                           for j in range(4)]
                    for p in range(HP):
                        for j in range(4):
                            oc = och * 4 + j
                            nc.tensor.matmul(
                                pss[j],
                                wo_t[p][:, oc * 128:(oc + 1) * 128],
                                avts[p][:, g0:g0 + gsz],
                                start=(p == 0), stop=(p == HP - 1))
                    for j in range(4):
                        oc = och * 4 + j
                        obias = bo_t[:, oc:oc + 1] \
                            if bo_t is not None else 0.0
                        nc.scalar.activation(
                            out=attnT[oc][:, g0:g0 + gsz], in_=pss[j],
                            func=AF.Identity, bias=obias)

                def ln2_rows(rb):
                    r = rb * 128
                    xo = p3sb.tile([128, C], DT, tag="xo", name="xo")
                    nc.sync.dma_start(out=xo, in_=xk[r:r + 128, :])
                    x2 = p3sb.tile([128, C], DT, tag="x2", name="x2")
                    for oc in range(NCCH):
                        tp = p3tp.tile([128, 128], MT, tag="tp", name="tp")
                        nc.tensor.transpose(
                            tp, attnT[oc][:, rb * 128:(rb + 1) * 128],
                            identity_m)
                        nc.vector.tensor_tensor(
                            out=x2[:, oc * 128:(oc + 1) * 128], in0=tp,
                            in1=xo[:, oc * 128:(oc + 1) * 128],
                            op=OP.add)
                    nc.sync.dma_start(out=x2_d[r:r + 128, :], in_=x2)
                    ln_m = layernorm(p3sb, p3st, x2, ln2g_t, ln2b_t,
                                     "ln2_gb" in skip)
                    for c in range(NCCH):
                        tp = p3tp.tile([128, 128], MT, tag="tp",
                                       name="tpm")
                        nc.tensor.transpose(
                            tp, ln_m[:, c * 128:(c + 1) * 128],
                            identity_m)
                        nc.vector.tensor_copy(
                            out=ln2T[:, c, r:r + 128], in_=tp)

                def fc_pass(gi, inject=()):
                    g0, gsz = own_groups[gi]
                    for fcg in range(NFG):
                        nfl = min(4, NF - fcg * 4)
                        wfs = []
                        for c in range(NCCH):
                            w = p4w.tile([128, 512], MT, tag=f"wf{c}",
                                         name=f"wf{c}")
                            nc.sync.dma_start(
                                out=w[:, 0:128 * nfl],
                                in_=W_fc[c * 128:(c + 1) * 128,
                                         fcg * 512:fcg * 512 + 128 * nfl])
                            wfs.append(w)
                        for fl in range(nfl):
                            fc = fcg * 4 + fl
                            fls = slice(fl * 128, (fl + 1) * 128)
                            ps = p4ps.tile([128, gsz], DT, tag="fps",
                                           name="fps")
                            for c in range(NCCH):
                                nc.tensor.matmul(
                                    ps, wfs[c][:, fls],
                                    ln2T[:, c, g0:g0 + gsz],
                                    start=(c == 0), stop=(c == NCCH - 1))
                            gbias = bfc_t[:, fc:fc + 1] \
                                if bfc_t is not None else 0.0
                            nc.scalar.activation(
                                out=h1T[:, fc, g0:g0 + gsz], in_=ps,
                                func=AF.Gelu_apprx_tanh, bias=gbias)
                        if fcg in inject:
                            ln2_rows(inject[fcg])

                # interleave: oproj passes hide LN2 of rows 0:512; fc
                # weight-groups hide LN2 of rows 512:1024
                oproj_pass(0, 0)
                oproj_pass(0, 1)
                ln2_rows(0)
                oproj_pass(1, 0)
                ln2_rows(1)
                oproj_pass(1, 1)
                ln2_rows(2)
                ln2_rows(3)
                fc_pass(0, inject={0: 4, 1: 5, 2: 6, 3: 7})
                fc_pass(1)

                NOC2 = (NCCH + 1) // 2
                for ocp in range(NOC2):
                    nol = min(2, NCCH - ocp * 2)
                    pss = {}
                    for ol in range(nol):
                        for gi in range(len(own_groups)):
                            pss[ol, gi] = p3ps.tile(
                                [128, own_groups[gi][1]], DT,
                                tag=f"ps{ol * 2 + gi}",
                                name=f"ps{ol * 2 + gi}")
                    for c2 in range(NF):
                        w = p4wp.tile([128, 256], MT, tag="wp", name="wp")
                        nc.sync.dma_start(
                            out=w[:, 0:128 * nol],
                            in_=W_proj[c2 * 128:(c2 + 1) * 128,
                                       ocp * 256:ocp * 256 + 128 * nol])
                        for ol in range(nol):
                            for gi, (g0, gsz) in enumerate(own_groups):
                                nc.tensor.matmul(
                                    pss[ol, gi],
                                    w[:, ol * 128:(ol + 1) * 128],
                                    h1T[:, c2, g0:g0 + gsz],
                                    start=(c2 == 0), stop=(c2 == NF - 1))
                    h2s = {}
                    for ol in range(nol):
                        oc = ocp * 2 + ol
                        h2s[ol] = p4h2.tile([128, OWN], MT, tag=f"h2_{ol}",
                                            name=f"h2_{ol}")
                        for gi, (g0, gsz) in enumerate(own_groups):
                            pbias = bpr_t[:, oc:oc + 1] \
                                if bpr_t is not None else 0.0
                            nc.scalar.activation(
                                out=h2s[ol][:, g0:g0 + gsz],
                                in_=pss[ol, gi],
                                func=AF.Identity, bias=pbias)
                    # residual + transpose back, column strip of this ocp;
                    # overlaps the next ocp's proj matmuls on the PE
                    cw = 128 * nol
                    for rb in range(OWN // 128):
                        r = rb * 128
                        x2t = p3sb.tile([128, 256], DT, tag="x2t",
                                        name="x2t")
                        nc.sync.dma_start(
                            out=x2t[:, 0:cw],
                            in_=x2_d[r:r + 128,
                                     ocp * 256:ocp * 256 + cw])
                        outt = p3sb.tile([128, 256], DT, tag="outt",
                                         name="outt")
                        for ol in range(nol):
                            tp = p3tp.tile([128, 128], MT, tag="tp",
                                           name="tp")
                            nc.tensor.transpose(
                                tp, h2s[ol][:, rb * 128:(rb + 1) * 128],
                                identity_m)
                            nc.vector.tensor_tensor(
                                out=outt[:, ol * 128:(ol + 1) * 128],
                                in0=tp,
                                in1=x2t[:, ol * 128:(ol + 1) * 128],
                                op=OP.add)
                        nc.sync.dma_start(
                            out=out[r:r + 128,
                                    ocp * 256:ocp * 256 + cw],
                            in_=outt[:, 0:cw])

    nc.compile()
    return nc


# ---------------------------------------------------------------------------
# host-side sharding
# ---------------------------------------------------------------------------

def detect_skips(inputs):
    def z(*ks):
        return all(not np.asarray(inputs[k]).any() for k in ks)
    skips = []
    if z("bq", "bk", "bv"):
        skips.append("qkv_bias")
    if z("bo"):
        skips.append("o_bias")
    if z("b_fc"):
        skips.append("fc_bias")
    if z("b_proj"):
        skips.append("proj_bias")
    if np.all(np.asarray(inputs["ln1_g"]) == 1.0) and z("ln1_b"):
        skips.append("ln1_gb")
    if np.all(np.asarray(inputs["ln2_g"]) == 1.0) and z("ln2_b"):
        skips.append("ln2_gb")
    return tuple(skips)


def shard_inputs(inputs, T=2048, C=1024, n_batch=4, mm_dtype="bf16"):
    """Build per-core in_maps for the 8-core SPMD launch."""
    import ml_dtypes
    wdt = ml_dtypes.bfloat16 if mm_dtype == "bf16" else np.float32
    f8 = ml_dtypes.float8_e4m3
    NB = T // 128
    NOB = NB // 2
    x = np.asarray(inputs["x"], np.float32)
    shared = {}
    for k in ("Wq", "Wk", "Wv", "Wo", "bq", "bk", "bv", "bo",
              "ln1_g", "ln1_b", "ln2_g", "ln2_b",
              "W_fc", "b_fc", "W_proj", "b_proj"):
        arr = np.asarray(inputs[k], np.float32)
        if k[0] == "W":
            arr = arr.astype(wdt)
        shared[k] = np.ascontiguousarray(arr)
    in_maps = []
    for b in range(n_batch):
        xb = x[b].reshape(NB, 128, C)
        for h in range(2):
            perm = [2 * j + h for j in range(NOB)] + \
                   [2 * j + (1 - h) for j in range(NOB)]
            xkp = np.ascontiguousarray(xb[perm].reshape(T, C))
            # multiplicative 0/1 parity masks for kv-blocks NOB+2g (slot 0)
            # and NOB+2g+1 (slot 1)
            mqa = np.ones((128, 2, 256), np.float32)
            if h == 0:
                mqa[:, 0, 0:128] = 0.0
                mqa[:, 1, :] = 0.0
            else:
                mqa[:, 1, 0:128] = 0.0
            m = dict(shared)
            m["xk"] = xkp
            m["maskq"] = mqa.astype(wdt)
            in_maps.append(m)
    return in_maps


def unshard_output(results, T=2048, C=1024, n_batch=4):
    NB = T // 128
    NOB = NB // 2
    out = np.empty((n_batch, T, C), np.float32)
    ci = 0
    for b in range(n_batch):
        for h in range(2):
            o = results[ci]["out"].reshape(NOB, 128, C)
            for i in range(NOB):
                g = 2 * i + h
                out[b, g * 128:(g + 1) * 128, :] = o[i]
            ci += 1
    return out


_CACHE = {}
_LOCK = threading.Lock()


def _get_program(T, C, H, skip):
    key = (T, C, H, skip)
    with _LOCK:
        if key not in _CACHE:
            _CACHE[key] = build_block_program(T=T, C=C, H=H, skip=skip)
        return _CACHE[key]


def run(inputs, trace=False, **kw):
    x = np.asarray(inputs["x"])
    B, T, C = x.shape
    H = 16
    skip = detect_skips(inputs)
    nc = _get_program(T, C, H, skip)
    in_maps = shard_inputs(inputs, T=T, C=C, n_batch=B)
    res = bass_utils.run_bass_kernel_spmd(
        nc, in_maps, core_ids=list(range(8)), trace=trace, **kw)
    return unshard_output(res.results, T=T, C=C, n_batch=B), res


def kernel(**inputs):
    return run(inputs)[0]



# revision 33
# speedup vs baseline: 1.0607x; 1.0533x over previous
"""Trainium2 Bass kernel for a dense transformer block (LN->attn->LN->MLP).

Sharding: 8 cores = (batch b in 0..3, parity h in 0..1). Core (b,h) owns the
interleaved 128-row q-blocks {h, h+2, ...} of batch b.  Host permutes the
batch's rows so the core's own blocks come first; causal structure is then
identical on every core (uniform SPMD program): own q-block i attends to
permuted kv-blocks [0..i] (own parity, triangular mask on block i) and
[NOB..NOB+i] (other parity; parity-dependent masking supplied as per-core
mask data).

Attention (v3): q-blocks processed in groups of 256 columns; S^T[kv,q] tiles
for two kv-blocks share one PSUM bank and one ACT exp; AV computes
av'^T = V'^T @ P^T (lhsT = V' with appended ones column -> softmax sums in
row 64), then a short transpose chain normalizes and re-transposes per
128-q-half.  All matmuls are bf16 (fp32 matmul runs 2-pass LOW_HIGH);
accumulation stays fp32 in PSUM, LN/softmax/residual arithmetic fp32.
"""

import math
import threading
from contextlib import ExitStack

import numpy as np

import concourse.bass as bass
import concourse.mybir as mybir
import concourse.tile as tile
from concourse import bacc, bass_utils
from concourse.masks import (make_identity, make_lower_triangular,
                             make_upper_triangular)

AF = mybir.ActivationFunctionType
OP = mybir.AluOpType
DT = mybir.dt.float32
BF = mybir.dt.bfloat16
F8 = mybir.dt.float8e4
PM = mybir.MatmulPerfMode
W8_SCALE = 64.0

LN_EPS = 1e-5
MASK_VAL = -30000.0



def build_block_program(T=2048, C=1024, H=16, gelu_mode="hw",
                        mm_dtype="bf16", skip=()):
    """Build the per-core SPMD Bass program. Returns compiled Bacc.

    skip: subset of {"qkv_bias","o_bias","fc_bias","proj_bias","ln1_gb",
    "ln2_gb"} -- ops elided because the host verified the params are
    identity (zero bias / unit gain).
    """
    D = 64
    GELU_C = math.sqrt(2.0 / math.pi)
    MT = BF if mm_dtype == "bf16" else DT
    NB = T // 128            # kv blocks (permuted)
    NOB = NB // 2            # own q-blocks
    NOG = NOB // 2           # own q-groups (256 rows)
    OWN = NOB * 128          # own rows
    NCCH = C // 128          # feature chunks
    F = 4 * C
    NF = F // 128
    HP = H // 2              # head pairs
    HPG = HP                 # single pass: all head pairs resident
    W1 = HPG * 128           # qkv weight tile width
    BN_W = min(C, 512)       # bn_stats subgroup width
    NST = C // BN_W

    GSZ = min(512, OWN)
    kv_groups = [(g, min(512, T - g)) for g in range(0, T, 512)]
    own_groups = [(g, min(GSZ, OWN - g)) for g in range(0, OWN, GSZ)]

    nc = bacc.Bacc("TRN2", target_bir_lowering=False, debug=False)

    xk = nc.dram_tensor("xk", [T, C], DT, kind="ExternalInput")
    maskq = nc.dram_tensor("maskq", [128, 2, 256], MT, kind="ExternalInput")
    Wq = nc.dram_tensor("Wq", [C, C], F8, kind="ExternalInput")
    Wk = nc.dram_tensor("Wk", [C, C], F8, kind="ExternalInput")
    Wv = nc.dram_tensor("Wv", [C, C], F8, kind="ExternalInput")
    Wo = nc.dram_tensor("Wo", [C, C], MT, kind="ExternalInput")
    bq = nc.dram_tensor("bq", [C], DT, kind="ExternalInput")
    bk = nc.dram_tensor("bk", [C], DT, kind="ExternalInput")
    bv = nc.dram_tensor("bv", [C], DT, kind="ExternalInput")
    bo = nc.dram_tensor("bo", [C], DT, kind="ExternalInput")
    ln1_g = nc.dram_tensor("ln1_g", [C], DT, kind="ExternalInput")
    ln1_b = nc.dram_tensor("ln1_b", [C], DT, kind="ExternalInput")
    ln2_g = nc.dram_tensor("ln2_g", [C], DT, kind="ExternalInput")
    ln2_b = nc.dram_tensor("ln2_b", [C], DT, kind="ExternalInput")
    W_fc = nc.dram_tensor("W_fc", [C, F], MT, kind="ExternalInput")
    b_fc = nc.dram_tensor("b_fc", [F], DT, kind="ExternalInput")
    W_proj = nc.dram_tensor("W_proj", [F, C], MT, kind="ExternalInput")
    b_proj = nc.dram_tensor("b_proj", [C], DT, kind="ExternalInput")
    out = nc.dram_tensor("out", [OWN, C], DT, kind="ExternalOutput")

    with tile.TileContext(nc) as tc:
        with ExitStack() as es0:
            consts = es0.enter_context(tc.tile_pool(name="consts", bufs=1))
            persist = es0.enter_context(tc.tile_pool(name="persist", bufs=1))
            dram = es0.enter_context(
                tc.tile_pool(name="dram", bufs=1, space="DRAM"))
            identity_m = consts.tile([128, 128], MT)
            make_identity(nc, identity_m)
            identity = consts.tile([128, 128], DT)
            make_identity(nc, identity)
            # multiplicative diag mask for own-parity block pair (2g, 2g+1):
            # [triu1 | ones | zeros | triu1] over S^T tiles [kv, q]
            mD = consts.tile([128, 512], MT)
            make_upper_triangular(nc, mD[:, 0:128], val=1.0, diag=True)
            nc.gpsimd.memset(mD[:, 128:256], 1.0)
            nc.gpsimd.memset(mD[:, 256:384], 0.0)
            make_upper_triangular(nc, mD[:, 384:512], val=1.0, diag=True)
            # per-core multiplicative parity masks (0/1), [slot0|slot1]
            mq = consts.tile([128, 512], MT)
            nc.gpsimd.dma_start(out=mq, in_=maskq[:, :, :])
            eps_t = consts.tile([128, 1], DT)
            nc.vector.memset(eps_t, LN_EPS)
            # bf16 ones row (softmax-denominator outer-product broadcast)
            onesb = consts.tile([1, 65], MT)
            nc.vector.memset(onesb, 1.0)
            # PE warmup: keep the systolic array busy through the initial
            # DMA window so the HAM clock gate opens before real matmuls
            wz = consts.tile([128, 512], MT)
            nc.gpsimd.memset(wz, 0.0)
            with ExitStack() as eswu:
                wup = eswu.enter_context(
                    tc.tile_pool(name="wup", bufs=1, space="PSUM"))
                for _ in range(40):
                    wps = wup.tile([128, 512], DT, tag="wps", name="wps")
                    nc.tensor.matmul(wps, identity_m, wz, start=True,
                                     stop=True)

            def bcast_tile(vec):
                t = consts.tile([128, C], DT, tag=f"bc_{vec.name}",
                                name=f"bc_{vec.name}")
                src = bass.AP(tensor=vec, offset=0, ap=[[0, 128], [1, C]])
                nc.gpsimd.dma_start(out=t, in_=src)
                return t

            ln1g_t = bcast_tile(ln1_g) if "ln1_gb" not in skip else None
            ln1b_t = bcast_tile(ln1_b) if "ln1_gb" not in skip else None
            ln2g_t = bcast_tile(ln2_g) if "ln2_gb" not in skip else None
            ln2b_t = bcast_tile(ln2_b) if "ln2_gb" not in skip else None

            def chunk_tile(vec, n):
                t = consts.tile([128, n], DT, tag=f"ck_{vec.name}",
                                name=f"ck_{vec.name}")
                nc.gpsimd.dma_start(
                    out=t, in_=vec.ap().rearrange("(a p) -> p a", p=128))
                return t

            qkv_bias = "qkv_bias" not in skip
            bq_t = chunk_tile(bq, NCCH) if qkv_bias else None
            bvb_t = bcast_tile(bv) if qkv_bias else None
            bk_t = chunk_tile(bk, NCCH) if qkv_bias else None
            bv_t = chunk_tile(bv, NCCH) if qkv_bias else None
            bo_t = chunk_tile(bo, NCCH) if "o_bias" not in skip else None
            bfc_t = chunk_tile(b_fc, NF) if "fc_bias" not in skip else None
            bpr_t = chunk_tile(b_proj, NCCH) \
                if "proj_bias" not in skip else None

            qT_d = dram.tile([HP, 128, OWN], MT, tag="qT", name="qT_d")
            avT_d = dram.tile([HP, 128, OWN], MT, tag="avT", name="avT_d")
            x2_d = dram.tile([OWN, C], DT, tag="x2", name="x2_d")

            def layernorm(pool, spool, xt, g_t, b_t, skip_gb):
                """LN of xt [128,C] f32 -> new MT tile."""
                stats = spool.tile([128, NST, 6], DT, tag="stats",
                                   name="stats")
                mv = spool.tile([128, 2], DT, tag="mv", name="mv")
                for s in range(NST):
                    nc.vector.bn_stats(out=stats[:, s, :],
                                       in_=xt[:, s * BN_W:(s + 1) * BN_W])
                nc.vector.bn_aggr(out=mv, in_=stats)
                rstd = spool.tile([128, 1], DT, tag="rstd", name="rstd")
                nc.scalar.activation(out=rstd, in_=mv[:, 1:2],
                                     func=AF.Sqrt, bias=eps_t[:, :])
                nc.vector.reciprocal(out=rstd, in_=rstd)
                ln_m = pool.tile([128, C], MT, tag="ln_m", name="ln_m")
                if skip_gb:
                    nc.vector.tensor_scalar(
                        out=ln_m, in0=xt, scalar1=mv[:, 0:1], scalar2=rstd,
                        op0=OP.subtract, op1=OP.mult)
                else:
                    ln = pool.tile([128, C], DT, tag="ln", name="ln")
                    nc.vector.tensor_scalar(
                        out=ln, in0=xt, scalar1=mv[:, 0:1], scalar2=rstd,
                        op0=OP.subtract, op1=OP.mult)
                    nc.vector.tensor_tensor(out=ln, in0=ln, in1=g_t,
                                            op=OP.mult)
                    nc.vector.tensor_tensor(out=ln_m, in0=ln, in1=b_t,
                                            op=OP.add)
                return ln_m

            # ===== Phases 1+2 per head-group =====
            for hg in range(HP // HPG):
                pairs = list(range(hg * HPG, (hg + 1) * HPG))
                hgs = slice(hg * W1, (hg + 1) * W1)
                with ExitStack() as es1:
                    kvp = es1.enter_context(tc.tile_pool(name="kvp", bufs=1))
                    KT = {p: kvp.tile([128, T], MT, tag=f"kt{p}",
                                      name=f"kt{p}") for p in pairs}
                    # V in [kv, head, dim|ones] layout, built directly by
                    # kv-block-major matmuls (lnT stationary, Wv moving) --
                    # no per-head transposes
                    VPA = kvp.tile([128, NB, H, 65], MT, tag="vpa",
                                   name="vpa")
                    nc.gpsimd.memset(VPA[:, :, :, 64:65], 1.0)
                    es1b = es1.enter_context(ExitStack())
                    p1sb = es1b.enter_context(
                        tc.tile_pool(name="p1sb", bufs=2))
                    p1st = es1b.enter_context(
                        tc.tile_pool(name="p1st", bufs=8))
                    p1lt = es1b.enter_context(
                        tc.tile_pool(name="p1lt", bufs=2))
                    p1w = es1b.enter_context(
                        tc.tile_pool(name="p1w", bufs=1))
                    p1ev = es1b.enter_context(
                        tc.tile_pool(name="p1ev", bufs=2))
                    # group-0-only pools: closed before attention opens so
                    # their PSUM banks and the q/v weights free up
                    es1c = es1b.enter_context(ExitStack())
                    p1ps = es1c.enter_context(
                        tc.tile_pool(name="p1ps", bufs=1, space="PSUM"))
                    p1wv = es1c.enter_context(
                        tc.tile_pool(name="p1wv", bufs=1))
                    p1vp = es1c.enter_context(
                        tc.tile_pool(name="p1vp", bufs=1, space="PSUM"))
                    p1tp = es1c.enter_context(
                        tc.tile_pool(name="p1tp", bufs=2, space="PSUM"))
                    NCP = NCCH // 2
                    wts = {}
                    for nm, Wt, pool in (("k", Wk, p1w), ("q", Wq, p1wv),
                                         ("v", Wv, p1wv)):
                        for c2 in range(NCP):
                            w = pool.tile([128, 2, W1], F8,
                                          tag=f"w{nm}{c2}",
                                          name=f"w{nm}{c2}")
                            nc.sync.dma_start(
                                out=w,
                                in_=Wt[c2 * 256:(c2 + 1) * 256, hgs]
                                .rearrange("(two p) f -> p two f", p=128))
                            wts[nm, c2] = w
                    kv_pairs = [(g, min(1024, T - g))
                                for g in range(0, T, 1024)]
                    lts_t = {}

                    def rb_pass(G0, Gsz):
                        ngb = Gsz // 128
                        lts = p1lt.tile([128, NCCH, Gsz], F8, tag="lts",
                                        name="lts")
                        lts_t[G0] = lts
                        for rb in range(ngb):
                            r = G0 + rb * 128
                            rs = slice(rb * 128, (rb + 1) * 128)
                            xt = p1sb.tile([128, C], DT, tag="xt",
                                           name="xt")
                            nc.sync.dma_start(out=xt, in_=xk[r:r + 128, :])
                            ln_m = layernorm(p1sb, p1st, xt, ln1g_t,
                                             ln1b_t, "ln1_gb" in skip)
                            for tq in range(2):
                                tpq = p1tp.tile([128, 4, 128], MT,
                                                tag="tpq", name="tpq")
                                for k in range(4):
                                    c = tq * 4 + k
                                    nc.tensor.transpose(
                                        tpq[:, k, :],
                                        ln_m[:, c * 128:(c + 1) * 128],
                                        identity_m)
                                nc.vector.tensor_copy(
                                    out=lts[:, tq * 4:tq * 4 + 4, rs],
                                    in_=tpq)
                            vps = p1vp.tile([128, 1024], DT, tag="vps",
                                            name="vps")
                            for c2 in range(NCP):
                                for half in range(2):
                                    hw = slice(half * 512,
                                               (half + 1) * 512)
                                    nc.tensor.matmul(
                                        vps[:, hw],
                                        lts[:, 2 * c2:2 * c2 + 2, rs],
                                        wts["v", c2][:, :, hw],
                                        start=(c2 == 0),
                                        stop=(c2 == NCP - 1),
                                        perf_mode=PM.DoubleRow)
                            blk = G0 // 128 + rb
                            vpsr = vps.rearrange("p (h d) -> p h d", h=H)
                            if qkv_bias:
                                nc.vector.scalar_tensor_tensor(
                                    out=VPA[:, blk, :, 0:64], in0=vpsr,
                                    scalar=1.0 / W8_SCALE,
                                    in1=bvb_t.rearrange(
                                        "p (h d) -> p h d", h=H),
                                    op0=OP.mult, op1=OP.add)
                            else:
                                nc.vector.tensor_scalar_mul(
                                    out=VPA[:, blk, :, 0:64], in0=vpsr,
                                    scalar1=1.0 / W8_SCALE)

                    def k_pair(G0, Gsz, p, with_q, kalloc=None):
                        subs = [(s, min(512, Gsz - s))
                                for s in range(0, Gsz, 512)]
                        lts = lts_t[G0]
                        pl = (p - hg * HPG) * 128
                        pls = slice(pl, pl + 128)
                        if kalloc is None:
                            def kalloc(si):
                                return p1ps.tile([128, 512], DT,
                                                 tag=f"ps{si}",
                                                 name=f"ps{si}")
                        for si, (s0, ssz) in enumerate(subs):
                            ps = kalloc(si)
                            for c2 in range(NCP):
                                nc.tensor.matmul(
                                    ps, wts["k", c2][:, :, pls],
                                    lts[:, 2 * c2:2 * c2 + 2,
                                        s0:s0 + ssz],
                                    start=(c2 == 0),
                                    stop=(c2 == NCP - 1),
                                    perf_mode=PM.DoubleRow)
                            g0 = G0 + s0
                            kbias = bk_t[:, p:p + 1] \
                                if qkv_bias else 0.0
                            nc.scalar.activation(
                                out=KT[p][:, g0:g0 + ssz], in_=ps,
                                func=AF.Identity, bias=kbias,
                                scale=1.0 / W8_SCALE)
                        if not with_q:
                            return
                        for si, (s0, ssz) in enumerate(subs):
                            ps = p1ps.tile([128, 512], DT, tag=f"ps{si}",
                                           name=f"ps{si}")
                            for c2 in range(NCP):
                                nc.tensor.matmul(
                                    ps, wts["q", c2][:, :, pls],
                                    lts[:, 2 * c2:2 * c2 + 2,
                                        s0:s0 + ssz],
                                    start=(c2 == 0),
                                    stop=(c2 == NCP - 1),
                                    perf_mode=PM.DoubleRow)
                            g0 = G0 + s0
                            qsb = p1ev.tile([128, ssz], MT,
                                            tag="qsb", name="qsb")
                            qbias = bq_t[:, p:p + 1] \
                                if qkv_bias else 0.0
                            nc.scalar.activation(
                                out=qsb, in_=ps,
                                func=AF.Identity, bias=qbias,
                                scale=1.0 / W8_SCALE)
                            nc.sync.dma_start(
                                out=qT_d[p, :, g0:g0 + ssz],
                                in_=qsb)

                    # phase A: own rows -- LN/V, then K+Q per pair
                    rb_pass(0, 1024)
                    for p in pairs:
                        k_pair(0, 1024, p, with_q=True)
                    # phase B: other-parity rows -- LN/V only; K per pair
                    # is deferred into the attention stream below
                    rb_pass(1024, 1024)
                    es1c.close()

                    # -------- attention, merged with group-1 K ------------
                    with ExitStack() as es2:
                        p2q = es2.enter_context(
                            tc.tile_pool(name="p2q", bufs=3))
                        p2pt = es2.enter_context(
                            tc.tile_pool(name="p2pt", bufs=14))
                        p2st = es2.enter_context(
                            tc.tile_pool(name="p2st", bufs=3))
                        p2sps = es2.enter_context(
                            tc.tile_pool(name="p2sps", bufs=2, space="PSUM"))
                        p2avp = es2.enter_context(
                            tc.tile_pool(name="p2avp", bufs=1, space="PSUM"))
                        p2bc = es2.enter_context(
                            tc.tile_pool(name="p2bc", bufs=1, space="PSUM"))

                        def norm_tail(st):
                            """Deferred per-(p,g) softmax normalize: by now
                            the DVE recip/cast of `st` has completed, so the
                            bc matmul doesn't stall the PE."""
                            avts2, rz16, p_, qs_ = st
                            bcp = p2bc.tile([65, 512], DT, tag="bc",
                                            name="bcp")
                            nc.tensor.matmul(
                                bcp, onesb, rz16, start=True, stop=True)
                            avn = p2st.tile([64, 512], MT, tag="avn",
                                            name="avn")
                            for h2 in range(2):
                                cs = slice(h2 * 256, h2 * 256 + 256)
                                nc.vector.tensor_tensor(
                                    out=avn[:, cs], in0=avts2[h2][0:64, :],
                                    in1=bcp[0:64, cs], op=OP.mult)
                            nc.sync.dma_start(
                                out=avT_d[p_, 0:64, qs_],
                                in_=avn[:, 0:256])
                            nc.sync.dma_start(
                                out=avT_d[p_, 64:128, qs_],
                                in_=avn[:, 256:512])

                        def emit_S(p, g):
                            """S matmuls + exps + masks for one (p, g)
                            segment; returns AV-phase state."""
                            qs = slice(g * 256, (g + 1) * 256)
                            blocks = list(range(2 * g + 2)) + \
                                [NOB + jj for jj in range(2 * g + 2)]
                            nquad = g + 1
                            D_i = 2 * g
                            O_i = 4 * g + 2
                            qt = p2q.tile([128, 256], MT, tag="qt",
                                          name="qt")
                            nc.sync.dma_start(out=qt, in_=qT_d[p, :, qs])
                            pts = {}
                            for qi in range(nquad):
                                quad = blocks[4 * qi:4 * qi + 4]
                                for h2 in range(2):
                                    hs = slice(h2 * 64, h2 * 64 + 64)
                                    sps = p2sps.tile([128, 1024], DT,
                                                     tag="sps", name="sps")
                                    for k, j in enumerate(quad):
                                        ss = slice(k * 256, k * 256 + 256)
                                        nc.tensor.matmul(
                                            sps[:, ss],
                                            KT[p][hs,
                                                  j * 128:(j + 1) * 128],
                                            qt[hs, :], start=True,
                                            stop=True)
                                    pt_sb = p2pt.tile([128, 1024], MT,
                                                      tag="pt", name="pt")
                                    nc.scalar.activation(
                                        out=pt_sb, in_=sps, func=AF.Exp,
                                        scale=0.125)
                                    if 4 * qi <= D_i < 4 * qi + 4:
                                        off = (D_i - 4 * qi) * 256
                                        sl = slice(off, off + 512)
                                        nc.vector.tensor_tensor(
                                            out=pt_sb[:, sl],
                                            in0=pt_sb[:, sl], in1=mD,
                                            op=OP.mult)
                                    if 4 * qi <= O_i < 4 * qi + 4:
                                        off = (O_i - 4 * qi) * 256
                                        sl = slice(off, off + 512)
                                        nc.vector.tensor_tensor(
                                            out=pt_sb[:, sl],
                                            in0=pt_sb[:, sl], in1=mq,
                                            op=OP.mult)
                                    pts[h2, qi] = pt_sb
                            return (pts, blocks, nquad, p, qs, g)

                        def emit_AV(st):
                            """AV matmuls + denominator prep; heads run
                            sequentially through one PSUM accumulator."""
                            pts, blocks, nquad, p, qs, g = st
                            nmm = 4 * g + 4
                            avts2 = {}
                            dent = p2st.tile([1, 512], DT, tag="dent",
                                             name="dent")
                            for h2 in range(2):
                                h = 2 * p + h2
                                avps = p2avp.tile([65, 256], DT,
                                                  tag=f"avps{h2}",
                                                  name=f"avps{h2}")
                                mi = 0
                                for qi in range(nquad):
                                    quad = blocks[4 * qi:4 * qi + 4]
                                    pt_sb = pts[h2, qi]
                                    for k, j in enumerate(quad):
                                        ss = slice(k * 256, k * 256 + 256)
                                        nc.tensor.matmul(
                                            avps, VPA[:, j, h, :],
                                            pt_sb[:, ss],
                                            start=(mi == 0),
                                            stop=(mi == nmm - 1))
                                        mi += 1
                                avts = p2st.tile([65, 256], DT,
                                                 tag=f"avts{h2}",
                                                 name=f"avts{h2}")
                                nc.vector.tensor_copy(out=avts, in_=avps)
                                # move denominator row to partition 0
                                # (custom DVE ops need base partition 0)
                                cs = slice(h2 * 256, h2 * 256 + 256)
                                nc.sync.dma_start(
                                    out=dent[0:1, cs],
                                    in_=avts[64:65, :])
                                avts2[h2] = avts
                            nc.vector.reciprocal_approx_fast(
                                out=dent, in_=dent)
                            rz16 = p2st.tile([1, 512], MT, tag="rz16",
                                             name="rz16")
                            nc.vector.tensor_copy(out=rz16, in_=dent)
                            return (avts2, rz16, p, qs)

                        # software pipeline: S(k+1) lands before AV(k) so
                        # the scalar engine always has score tiles; each
                        # pair's other-parity K slots in just ahead of its
                        # first segment and hides under the exp stream
                        def kalloc_bc(si):
                            return p2bc.tile([128, 512], DT, tag="bc",
                                             name="kps")

                        sched = [(p, g) for p in pairs for g in range(NOG)]
                        k_pair(1024, 1024, pairs[0], with_q=False,
                               kalloc=kalloc_bc)
                        av_state = None
                        norm_pending = None
                        for si_, (p, g) in enumerate(sched):
                            if g == 2 and p != pairs[-1]:
                                k_pair(1024, 1024, p + 1, with_q=False,
                                       kalloc=kalloc_bc)
                            for _ in range(1 + g if g else 0):
                                bft = p2bc.tile([65, 512], DT,
                                                tag="bc", name="bft")
                                nc.tensor.matmul(
                                    bft[0:64, :], identity_m[:, 0:64],
                                    wz, start=True, stop=True)
                            s_next = emit_S(p, g)
                            if norm_pending is not None:
                                norm_tail(norm_pending)
                            if av_state is not None:
                                norm_pending = emit_AV(av_state)
                            av_state = s_next
                        norm_tail(norm_pending)
                        norm_tail(emit_AV(av_state))

            # ===== Phases 3+4: oproj + LN2 + MLP, one pipelined scope ======
            # Emission order keeps the PE dense: oproj -> LN2 rows 0:512 ->
            # fc matmuls for cols 0:512 (LN2 rows 512:1024 run on DVE
            # underneath) -> fc cols 512:1024 -> proj -> output rows.
            ln2T = persist.tile([128, NCCH, OWN], MT, tag="l2t",
                                name="ln2T")
            NFG = (NF + 3) // 4      # fc chunk groups of 4
            with ExitStack() as es3:
                p3av = es3.enter_context(tc.tile_pool(name="p3av", bufs=1))
                p3w = es3.enter_context(tc.tile_pool(name="p3w", bufs=1))
                p3at = es3.enter_context(tc.tile_pool(name="p3at", bufs=1))
                p3sb = es3.enter_context(tc.tile_pool(name="p3sb", bufs=2))
                p3st = es3.enter_context(tc.tile_pool(name="p3st", bufs=8))
                p4h1 = es3.enter_context(tc.tile_pool(name="p4h1", bufs=1))
                p4w = es3.enter_context(tc.tile_pool(name="p4w", bufs=2))
                p4wp = es3.enter_context(tc.tile_pool(name="p4wp", bufs=12))
                p4h2 = es3.enter_context(tc.tile_pool(name="p4h2", bufs=2))
                p3ps = es3.enter_context(
                    tc.tile_pool(name="p3ps", bufs=1, space="PSUM"))
                p4ps = es3.enter_context(
                    tc.tile_pool(name="p4ps", bufs=2, space="PSUM"))
                p3tp = es3.enter_context(
                    tc.tile_pool(name="p3tp", bufs=2, space="PSUM"))
                # PE warm bridge over the avts/wo load window
                for _ in range(8):
                    bps = p3ps.tile([128, 512], DT, tag="ps0", name="bps")
                    nc.tensor.matmul(bps, identity_m, wz, start=True,
                                     stop=True)
                wo_t = []
                for p in range(HP):
                    w = p3w.tile([128, C], MT, tag=f"wo{p}", name=f"wo{p}")
                    nc.sync.dma_start(out=w,
                                      in_=Wo[p * 128:(p + 1) * 128, :])
                    wo_t.append(w)
                avts = [p3av.tile([128, OWN], MT, tag=f"avt{p}",
                                  name=f"avt{p}")
                        for p in range(HP)]
                for p in range(HP):
                    nc.sync.dma_start(out=avts[p], in_=avT_d[p, :, :])
                attnT = [p3at.tile([128, OWN], MT, tag=f"atT{oc}",
                                   name=f"atT{oc}")
                         for oc in range(NCCH)]
                h1T = p4h1.tile([128, NF, OWN], MT, tag="h1",
                                name="h1T")

                def oproj_pass(gi, och):
                    g0, gsz = own_groups[gi]
                    pss = [p3ps.tile([128, gsz], DT, tag=f"ps{j}",
                                     name=f"ps{j}")
                           for j in range(4)]
                    for p in range(HP):
                        for j in range(4):
                            oc = och * 4 + j
                            nc.tensor.matmul(
                                pss[j],
                                wo_t[p][:, oc * 128:(oc + 1) * 128],
                                avts[p][:, g0:g0 + gsz],
                                start=(p == 0), stop=(p == HP - 1))
                    for j in range(4):
                        oc = och * 4 + j
                        obias = bo_t[:, oc:oc + 1] \
                            if bo_t is not None else 0.0
                        nc.scalar.activation(
                            out=attnT[oc][:, g0:g0 + gsz], in_=pss[j],
                            func=AF.Identity, bias=obias)

                def ln2_rows(rb):
                    r = rb * 128
                    xo = p3sb.tile([128, C], DT, tag="xo", name="xo")
                    nc.sync.dma_start(out=xo, in_=xk[r:r + 128, :])
                    x2 = p3sb.tile([128, C], DT, tag="x2", name="x2")
                    for oc in range(NCCH):
                        tp = p3tp.tile([128, 128], MT, tag="tp", name="tp")
                        nc.tensor.transpose(
                            tp, attnT[oc][:, rb * 128:(rb + 1) * 128],
                            identity_m)
                        nc.vector.tensor_tensor(
                            out=x2[:, oc * 128:(oc + 1) * 128], in0=tp,
                            in1=xo[:, oc * 128:(oc + 1) * 128],
                            op=OP.add)
                    nc.sync.dma_start(out=x2_d[r:r + 128, :], in_=x2)
                    ln_m = layernorm(p3sb, p3st, x2, ln2g_t, ln2b_t,
                                     "ln2_gb" in skip)
                    for c in range(NCCH):
                        tp = p3tp.tile([128, 128], MT, tag="tp",
                                       name="tpm")
                        nc.tensor.transpose(
                            tp, ln_m[:, c * 128:(c + 1) * 128],
                            identity_m)
                        nc.vector.tensor_copy(
                            out=ln2T[:, c, r:r + 128], in_=tp)

                def fc_pass(gi, inject=()):
                    g0, gsz = own_groups[gi]
                    for fcg in range(NFG):
                        nfl = min(4, NF - fcg * 4)
                        wfs = []
                        for c in range(NCCH):
                            w = p4w.tile([128, 512], MT, tag=f"wf{c}",
                                         name=f"wf{c}")
                            nc.sync.dma_start(
                                out=w[:, 0:128 * nfl],
                                in_=W_fc[c * 128:(c + 1) * 128,
                                         fcg * 512:fcg * 512 + 128 * nfl])
                            wfs.append(w)
                        for fl in range(nfl):
                            fc = fcg * 4 + fl
                            fls = slice(fl * 128, (fl + 1) * 128)
                            ps = p4ps.tile([128, gsz], DT, tag="fps",
                                           name="fps")
                            for c in range(NCCH):
                                nc.tensor.matmul(
                                    ps, wfs[c][:, fls],
                                    ln2T[:, c, g0:g0 + gsz],
                                    start=(c == 0), stop=(c == NCCH - 1))
                            gbias = bfc_t[:, fc:fc + 1] \
                                if bfc_t is not None else 0.0
                            nc.scalar.activation(
                                out=h1T[:, fc, g0:g0 + gsz], in_=ps,
                                func=AF.Gelu_apprx_tanh, bias=gbias)
                        if fcg in inject:
                            ln2_rows(inject[fcg])

                # interleave: oproj passes hide LN2 of rows 0:512; fc
                # weight-groups hide LN2 of rows 512:1024
                oproj_pass(0, 0)
                oproj_pass(0, 1)
                ln2_rows(0)
                oproj_pass(1, 0)
                ln2_rows(1)
                oproj_pass(1, 1)
                ln2_rows(2)
                ln2_rows(3)
                fc_pass(0, inject={0: 4, 1: 5, 2: 6, 3: 7})
                fc_pass(1)

                NOC2 = (NCCH + 1) // 2
                for ocp in range(NOC2):
                    nol = min(2, NCCH - ocp * 2)
                    pss = {}
                    for ol in range(nol):
                        for gi in range(len(own_groups)):
                            pss[ol, gi] = p3ps.tile(
                                [128, own_groups[gi][1]], DT,
                                tag=f"ps{ol * 2 + gi}",
                                name=f"ps{ol * 2 + gi}")
                    for c2 in range(NF):
                        w = p4wp.tile([128, 256], MT, tag="wp", name="wp")
                        nc.sync.dma_start(
                            out=w[:, 0:128 * nol],
                            in_=W_proj[c2 * 128:(c2 + 1) * 128,
                                       ocp * 256:ocp * 256 + 128 * nol])
                        for ol in range(nol):
                            for gi, (g0, gsz) in enumerate(own_groups):
                                nc.tensor.matmul(
                                    pss[ol, gi],
                                    w[:, ol * 128:(ol + 1) * 128],
                                    h1T[:, c2, g0:g0 + gsz],
                                    start=(c2 == 0), stop=(c2 == NF - 1))
                    h2s = {}
                    for ol in range(nol):
                        oc = ocp * 2 + ol
                        h2s[ol] = p4h2.tile([128, OWN], MT, tag=f"h2_{ol}",
                                            name=f"h2_{ol}")
                        for gi, (g0, gsz) in enumerate(own_groups):
                            pbias = bpr_t[:, oc:oc + 1] \
                                if bpr_t is not None else 0.0
                            nc.scalar.activation(
                                out=h2s[ol][:, g0:g0 + gsz],
                                in_=pss[ol, gi],
                                func=AF.Identity, bias=pbias)
                    # residual + transpose back, column strip of this ocp;
                    # overlaps the next ocp's proj matmuls on the PE
                    cw = 128 * nol
                    for rb in range(OWN // 128):
                        r = rb * 128
                        x2t = p3sb.tile([128, 256], DT, tag="x2t",
                                        name="x2t")
                        nc.sync.dma_start(
                            out=x2t[:, 0:cw],
                            in_=x2_d[r:r + 128,
                                     ocp * 256:ocp * 256 + cw])
                        outt = p3sb.tile([128, 256], DT, tag="outt",
                                         name="outt")
                        for ol in range(nol):
                            tp = p3tp.tile([128, 128], MT, tag="tp",
                                           name="tp")
                            nc.tensor.transpose(
                                tp, h2s[ol][:, rb * 128:(rb + 1) * 128],
                                identity_m)
                            nc.vector.tensor_tensor(
                                out=outt[:, ol * 128:(ol + 1) * 128],
                                in0=tp,
                                in1=x2t[:, ol * 128:(ol + 1) * 128],
                                op=OP.add)
                        nc.sync.dma_start(
                            out=out[r:r + 128,
                                    ocp * 256:ocp * 256 + cw],
                            in_=outt[:, 0:cw])

    nc.compile()
    return nc


# ---------------------------------------------------------------------------
# host-side sharding
# ---------------------------------------------------------------------------

def detect_skips(inputs):
    def z(*ks):
        return all(not np.asarray(inputs[k]).any() for k in ks)
    skips = []
    if z("bq", "bk", "bv"):
        skips.append("qkv_bias")
    if z("bo"):
        skips.append("o_bias")
    if z("b_fc"):
        skips.append("fc_bias")
    if z("b_proj"):
        skips.append("proj_bias")
    if np.all(np.asarray(inputs["ln1_g"]) == 1.0) and z("ln1_b"):
        skips.append("ln1_gb")
    if np.all(np.asarray(inputs["ln2_g"]) == 1.0) and z("ln2_b"):
        skips.append("ln2_gb")
    return tuple(skips)


def shard_inputs(inputs, T=2048, C=1024, n_batch=4, mm_dtype="bf16"):
    """Build per-core in_maps for the 8-core SPMD launch."""
    import ml_dtypes
    wdt = ml_dtypes.bfloat16 if mm_dtype == "bf16" else np.float32
    f8 = ml_dtypes.float8_e4m3
    NB = T // 128
    NOB = NB // 2
    x = np.asarray(inputs["x"], np.float32)
    shared = {}
    for k in ("Wq", "Wk", "Wv", "Wo", "bq", "bk", "bv", "bo",
              "ln1_g", "ln1_b", "ln2_g", "ln2_b",
              "W_fc", "b_fc", "W_proj", "b_proj"):
        arr = np.asarray(inputs[k], np.float32)
        if k in ("Wq", "Wk", "Wv"):
            arr = np.clip(arr * 64.0, -240.0, 240.0).astype(
                ml_dtypes.float8_e4m3)
        elif k[0] == "W":
            arr = arr.astype(wdt)
        shared[k] = np.ascontiguousarray(arr)
    in_maps = []
    for b in range(n_batch):
        xb = x[b].reshape(NB, 128, C)
        for h in range(2):
            perm = [2 * j + h for j in range(NOB)] + \
                   [2 * j + (1 - h) for j in range(NOB)]
            xkp = np.ascontiguousarray(xb[perm].reshape(T, C))
            # multiplicative 0/1 parity masks for kv-blocks NOB+2g (slot 0)
            # and NOB+2g+1 (slot 1)
            mqa = np.ones((128, 2, 256), np.float32)
            if h == 0:
                mqa[:, 0, 0:128] = 0.0
                mqa[:, 1, :] = 0.0
            else:
                mqa[:, 1, 0:128] = 0.0
            m = dict(shared)
            m["xk"] = xkp
            m["maskq"] = mqa.astype(wdt)
            in_maps.append(m)
    return in_maps


def unshard_output(results, T=2048, C=1024, n_batch=4):
    NB = T // 128
    NOB = NB // 2
    out = np.empty((n_batch, T, C), np.float32)
    ci = 0
    for b in range(n_batch):
        for h in range(2):
            o = results[ci]["out"].reshape(NOB, 128, C)
            for i in range(NOB):
                g = 2 * i + h
                out[b, g * 128:(g + 1) * 128, :] = o[i]
            ci += 1
    return out


_CACHE = {}
_LOCK = threading.Lock()


def _get_program(T, C, H, skip):
    key = (T, C, H, skip)
    with _LOCK:
        if key not in _CACHE:
            _CACHE[key] = build_block_program(T=T, C=C, H=H, skip=skip)
        return _CACHE[key]


def run(inputs, trace=False, **kw):
    x = np.asarray(inputs["x"])
    B, T, C = x.shape
    H = 16
    skip = detect_skips(inputs)
    nc = _get_program(T, C, H, skip)
    in_maps = shard_inputs(inputs, T=T, C=C, n_batch=B)
    res = bass_utils.run_bass_kernel_spmd(
        nc, in_maps, core_ids=list(range(8)), trace=trace, **kw)
    return unshard_output(res.results, T=T, C=C, n_batch=B), res


def kernel(**inputs):
    return run(inputs)[0]

